# revision 1
# baseline (speedup 1.0000x reference)
"""Dense transformer block on 8 TRN2 NeuronCores.

Sharding: data-parallel over batch (4 pairs) x Megatron tensor-parallel 2-way
within each pair (QKV/proj split over heads, MLP fc/cproj split over the 4096
hidden dim), with a pairwise AllReduce after the attention projection and
after the MLP projection.

Device layout is feature-major ("transposed"): activations live as
[d_model, tokens] so every matmul contracts along the partition dim with
naturally-laid-out weights and no on-device transposes. The host feeds x
pre-transposed (tile-packed) and re-assembles the output.

Attention: scores are computed transposed (S^T[k_pos, q_pos]); softmax needs
no max-subtraction (scores are O(1) by construction); the attention-forcing
reweight (w[k] for k >= idx) folds into the exp as a per-partition ln(w)
bias; the softmax denominator rides the attn@V matmul as a 65th ones-column
of V. Causal masking: future kpos tiles are not computed, diagonal 128x128
blocks get a triangular mask post-exp, diagonal-region matmuls are
column-narrowed.

The emission order is software-pipelined with a 2-stage skew so the PE always
has independent work while the AllReduces and LN stat round-trips are in
flight:  s1 = qkv+attn+proj+AR1-start, s2 = LN1+fc+cproj+AR2-start,
s3 = LN2+store;  order: s1(0) s1(1) s2(0) s1(2) s2(1) s3(0) ...
"""

import numpy as np
import ml_dtypes

import concourse.bacc as bacc
import concourse.mybir as mybir
import concourse.tile as tile
from concourse.bass_utils import run_bass_kernel_spmd

F32 = mybir.dt.float32
BF16 = mybir.dt.bfloat16
AF = mybir.ActivationFunctionType
OP = mybir.AluOpType

B, S, D, H, HD, FF = 4, 2048, 1024, 16, 64, 4096
N_CORES = 8
PAIRS = [[0, 1], [2, 3], [4, 5], [6, 7]]
CH = 512                 # tokens per pipeline chunk
NCH = S // CH            # 4
DT = D // 128            # 8 d-tiles
KT = S // 128            # 16 kpos tiles
HPC = H // 2             # heads per core (TP-2)
EPS = 1e-5
BF = ml_dtypes.bfloat16


def _build(use_bqk, use_bv, use_projb, use_cprojb, use_g1b1, use_g2b2):
    nc = bacc.Bacc("TRN2", target_bir_lowering=False, debug=False,
                   enable_asserts=True, num_devices=N_CORES)

    # tile-packed inputs: leading dim indexes [128, X] tiles, each contiguous
    xq = nc.dram_tensor("xq", [NCH * DT, 128, 512], F32, kind="ExternalInput")
    wqk = nc.dram_tensor("wqk", [16, 128, 512], BF16, kind="ExternalInput")
    bqk = nc.dram_tensor("bqk", [1024], F32, kind="ExternalInput")
    wv = nc.dram_tensor("wv", [8, 128, 512], BF16, kind="ExternalInput")
    bv = nc.dram_tensor("bv", [512], BF16, kind="ExternalInput")
    wproj = nc.dram_tensor("wproj", [8, 128, 512], BF16, kind="ExternalInput")
    projb = nc.dram_tensor("projb", [D], F32, kind="ExternalInput")
    wfc = nc.dram_tensor("wfc", [32, 128, 512], BF16, kind="ExternalInput")
    fcb = nc.dram_tensor("fcb", [2048], F32, kind="ExternalInput")
    wcproj = nc.dram_tensor("wcproj", [64, 128, 256], BF16,
                            kind="ExternalInput")
    cprojb = nc.dram_tensor("cprojb", [D], F32, kind="ExternalInput")
    g1 = nc.dram_tensor("g1", [D], F32, kind="ExternalInput")
    b1 = nc.dram_tensor("b1", [D], F32, kind="ExternalInput")
    g2 = nc.dram_tensor("g2", [D], F32, kind="ExternalInput")
    b2 = nc.dram_tensor("b2", [D], F32, kind="ExternalInput")
    lna = nc.dram_tensor("lna", [S], F32, kind="ExternalInput")
    tri = nc.dram_tensor("tri", [128, 128], BF16, kind="ExternalInput")
    # tile-packed output: [chunk*DT + dtile, 128, 512]; host reassembles
    out = nc.dram_tensor("out", [NCH * DT, 128, 512], F32,
                         kind="ExternalOutput")

    from contextlib import ExitStack
    with tile.TileContext(nc) as tc, ExitStack() as ctx:
        def pool(name, bufs, space="SBUF"):
            return ctx.enter_context(
                tc.tile_pool(name=name, bufs=bufs, space=space))

        const = pool("const", 1)
        wqk_p = pool("wqk_p", 8)
        wv_p = pool("wv_p", 8)
        wproj_p = pool("wproj_p", 4)
        wfc_p = pool("wfc_p", 8)
        wcproj_p = pool("wcproj_p", 18)
        xstage = pool("xstage", 1)
        xTb_p = pool("xTb_p", 8)
        qTb_p = pool("qTb_p", 4)
        pP = pool("pP", 3)
        attnTb_p = pool("attnTb_p", 4)
        den_p = pool("den_p", 1)
        den1_p = pool("den1_p", 1)
        tmp64_p = pool("tmp64_p", 1)
        recip_p = pool("recip_p", 1)
        recipb_p = pool("recipb_p", 2)
        arin_p = pool("arin_p", 2)
        art_p = pool("art_p", 8)
        cast_p = pool("cast_p", 2)
        sq_p = pool("sq_p", 2)
        strow_p = pool("strow_p", 1)
        bcast_p = pool("bcast_p", 2)
        nf_p = pool("nf_p", 2)
        xf2_p = pool("xf2_p", 2)
        nf2_p = pool("nf2_p", 2)
        nTb_p = pool("nTb_p", 9)
        gT_p = pool("gT_p", 16)
        mT_p = pool("mT_p", 2)
        mar_p = pool("mar_p", 8)
        hT_p = pool("hT_p", 2)
        psS = pool("psS", 2, "PSUM")
        psA = pool("psA", 2, "PSUM")
        psM = pool("psM", 2, "PSUM")
        psT = pool("psT", 2, "PSUM")
        dram = pool("dram", 4, "DRAM")

        # ---- persistent state + constants ----
        kt_sb = const.tile([128, 4 * S], BF16, name="kt_sb")
        kt_v = kt_sb[:].rearrange("p (r q) -> p r q", q=S)
        v_sb = const.tile([128, KT * 520], BF16, name="v_sb")
        v_v = v_sb[:].rearrange("p (t e) -> p t e", e=520)

        tri_sb = const.tile([128, 128], BF16, name="tri_sb")
        nc.sync.dma_start(out=tri_sb[:], in_=tri[:])
        lna_sb = const.tile([128, KT], F32, name="lna_sb")
        nc.sync.dma_start(out=lna_sb[:],
                          in_=lna.rearrange("(t p) -> p t", p=128))
        ones_col_b = const.tile([128, 1], BF16, name="ones_col_b")
        nc.vector.memset(ones_col_b[:], 1.0)
        eps_sb = const.tile([1, 1], F32, name="eps_sb")
        nc.vector.memset(eps_sb[:], EPS)
        fcb_sb = const.tile([128, 16], F32, name="fcb_sb")
        nc.sync.dma_start(out=fcb_sb[:],
                          in_=fcb.rearrange("(i p) -> p i", p=128))

        def vec8(name, t):
            sb = const.tile([128, DT], F32, name=name)
            nc.sync.dma_start(out=sb[:],
                              in_=t.rearrange("(i p) -> p i", p=128))
            return sb

        bqk_sb = vec8("bqk_sb", bqk) if use_bqk else None
        projb_sb = vec8("projb_sb", projb) if use_projb else None
        cprojb_sb = vec8("cprojb_sb", cprojb) if use_cprojb else None
        g1_sb = vec8("g1_sb", g1) if use_g1b1 else None
        b1_sb = vec8("b1_sb", b1) if use_g1b1 else None
        g2_sb = vec8("g2_sb", g2) if use_g2b2 else None
        b2_sb = vec8("b2_sb", b2) if use_g2b2 else None
        if use_bv:
            ones_row_b = const.tile([1, 128], BF16, name="ones_row_b")
            nc.vector.memset(ones_row_b[:], 1.0)
            bv_sb = const.tile([1, 512], BF16, name="bv_sb")
            nc.sync.dma_start(out=bv_sb[:],
                              in_=bv.rearrange("(o q) -> o q", o=1))

        def layernorm(src_t, out_mk, g_sb, b_sb, use_gb):
            """src_t: 8 f32 [128,512] tiles, scratched in place with t*rstd.
            out_mk(i, urstd_b, g_sb, b_sb, use_gb) writes the output tile."""
            ps_sumA = psT.tile([1, 512], F32, tag="pst", name="ps_sumA")
            ps_sumB = psT.tile([1, 512], F32, tag="pst", name="ps_sumB")
            for i in range(DT):
                tb = cast_p.tile([128, 512], BF16, name="tb")
                nc.vector.tensor_copy(tb[:], src_t[i][:])
                nc.tensor.matmul(ps_sumA[:], ones_col_b[:], tb[:],
                                 start=(i == 0), stop=(i == DT - 1))
                sqt = sq_p.tile([128, 512], BF16, name="sqt")
                nc.scalar.activation(sqt[:], src_t[i][:], AF.Square)
                nc.tensor.matmul(ps_sumB[:], ones_col_b[:], sqt[:],
                                 start=(i == 0), stop=(i == DT - 1))
            st = strow_p.tile([1, 3 * 512], F32, tag="st", name="st")
            sA, sB, sC = st[:, 0:512], st[:, 512:1024], st[:, 1024:1536]
            nc.scalar.activation(sA, ps_sumA[:], AF.Copy, scale=1.0 / D)  # u
            nc.scalar.activation(sB, ps_sumB[:], AF.Identity,
                                 bias=eps_sb[:], scale=1.0 / D)   # msq+eps
            nc.scalar.activation(sC, sA, AF.Square)               # u^2
            nc.vector.tensor_sub(sB, sB, sC)                      # var
            nc.vector.reciprocal(sC, sB)                          # 1/var
            nc.scalar.activation(sB, sC, AF.Sqrt)                 # rstd
            nc.vector.tensor_mul(sA, sA, sB)                      # u*rstd
            srd = dram.tile([2, 512], F32, tag="strow_d", name="srd")
            nc.sync.dma_start(out=srd[0:1, :], in_=sB)
            nc.sync.dma_start(out=srd[1:2, :], in_=sA)
            rstd_b = bcast_p.tile([128, 512], F32, name="rstd_b")
            nc.sync.dma_start(out=rstd_b[:],
                              in_=srd[0:1, :].partition_broadcast(128))
            urstd_b = bcast_p.tile([128, 512], F32, name="urstd_b")
            nc.sync.dma_start(out=urstd_b[:],
                              in_=srd[1:2, :].partition_broadcast(128))
            for i in range(DT):
                nc.vector.tensor_mul(src_t[i][:], src_t[i][:], rstd_b[:])
                out_mk(i, urstd_b, g_sb, b_sb, use_gb)

        state = {}

        # ================= stage 1: qkv + attention + proj + AR1 ============
        def s1(c):
            tok = slice(CH * c, CH * (c + 1))
            xTb_t = []
            for i in range(DT):
                xs = xstage.tile([128, 512], F32, name="xs")
                nc.sync.dma_start(out=xs[:], in_=xq[DT * c + i])
                xb = xTb_p.tile([128, 512], BF16, name="xb")
                nc.vector.tensor_copy(xb[:], xs[:])
                xTb_t.append(xb)

            qTb_t = []
            for cc in range(2):
                wt = []
                for d in range(DT):
                    w = wqk_p.tile([128, 512], BF16, name="w_qk")
                    nc.sync.dma_start(out=w[:], in_=wqk[8 * cc + d])
                    wt.append(w)
                for ct in range(4):
                    i = 4 * cc + ct
                    ps = psM.tile([128, 512], F32, tag="mm", name="ps_qk")
                    for d in range(DT):
                        nc.tensor.matmul(
                            ps[:], wt[d][:, 128 * ct:128 * (ct + 1)],
                            xTb_t[d][:], start=(d == 0), stop=(d == DT - 1))
                    if i < 4:
                        dest_t = qTb_p.tile([128, 512], BF16, name="qTb")
                        qTb_t.append(dest_t)
                        dest = dest_t[:]
                    else:
                        dest = kt_v[:, i - 4, tok]
                    if use_bqk:
                        nc.scalar.activation(dest, ps[:], AF.Identity,
                                             bias=bqk_sb[:, i:i + 1])
                    else:
                        nc.scalar.copy(dest, ps[:])
                yield

            wvt = []
            for d in range(DT):
                w = wv_p.tile([128, 512], BF16, name="w_v")
                nc.sync.dma_start(out=w[:], in_=wv[d])
                wvt.append(w)
            for tt in range(4):
                tg = 4 * c + tt
                ps = psM.tile([128, 512], F32, tag="mm", name="ps_v")
                for d in range(DT):
                    nc.tensor.matmul(
                        ps[:], xTb_t[d][:, 128 * tt:128 * (tt + 1)],
                        wvt[d][:], start=(d == 0),
                        stop=(d == DT - 1 and not use_bv))
                if use_bv:
                    nc.tensor.matmul(ps[:], ones_row_b[:], bv_sb[:],
                                     start=False, stop=True)
                v3 = v_v[:, tg, :].rearrange("p (h e) -> p h e", e=65)
                nc.vector.tensor_copy(v3[:, :, 0:64],
                                      ps[:].rearrange("p (h e) -> p h e", e=64))
                nc.vector.memset(v3[:, :, 64:65], 1.0)
            yield

            # ---- attention: head pairs (row-tiled concurrent score MMs) ----
            attnTb_t = [attnTb_p.tile([128, 512], BF16, tag="attnTb",
                                      name=f"attnTb{r}") for r in range(4)]
            den_t = den_p.tile([8, 512], F32, name="den")
            nt = 4 * (c + 1)
            for krt in range(4):
                h0, h1 = 2 * krt, 2 * krt + 1
                q0 = qTb_t[krt][0:64, :]
                q1 = qTb_t[krt][64:128, :]
                psa0 = psA.tile([65, 512], F32, tag="psa", name="psa0")
                psa1 = psA.tile([65, 512], F32, tag="psa", name="psa1")
                for t in range(nt):
                    j = t - 4 * c
                    qo = 128 * j if j >= 0 else 0
                    ks = kt_v[:, krt, 128 * t:128 * (t + 1)]
                    ps0 = psS.tile([128, 512], F32, tag="ps_s", name="ps0")
                    ps1 = psS.tile([128, 512], F32, tag="ps_s", name="ps1")
                    nc.tensor.matmul(ps0[:, qo:], ks[0:64, :], q0[:, qo:],
                                     start=True, stop=True)
                    nc.tensor.matmul(ps1[:, qo:], ks[64:128, :], q1[:, qo:],
                                     start=True, stop=True)
                    pt0 = pP.tile([128, 512], BF16, tag="pt", name="pt0")
                    pt1 = pP.tile([128, 512], BF16, tag="pt", name="pt1")
                    nc.scalar.activation(pt0[:, qo:], ps0[:, qo:], AF.Exp,
                                         bias=lna_sb[:, t:t + 1], scale=0.125)
                    nc.scalar.activation(pt1[:, qo:], ps1[:, qo:], AF.Exp,
                                         bias=lna_sb[:, t:t + 1], scale=0.125)
                    if j >= 0:
                        nc.vector.tensor_mul(pt0[:, qo:qo + 128],
                                             pt0[:, qo:qo + 128], tri_sb[:])
                        nc.vector.tensor_mul(pt1[:, qo:qo + 128],
                                             pt1[:, qo:qo + 128], tri_sb[:])
                    nc.tensor.matmul(psa0[:, qo:],
                                     v_v[:, t, 65 * h0:65 * (h0 + 1)],
                                     pt0[:, qo:], start=(t == 0),
                                     stop=(t == nt - 1))
                    nc.tensor.matmul(psa1[:, qo:],
                                     v_v[:, t, 65 * h1:65 * (h1 + 1)],
                                     pt1[:, qo:], start=(t == 0),
                                     stop=(t == nt - 1))
                for h, psa, koff in ((h0, psa0, 0), (h1, psa1, 64)):
                    d1 = den1_p.tile([65, 512], F32, tag="d1", name="d1")
                    nc.vector.tensor_copy(d1[64:65, :], psa[64:65, :])
                    nc.sync.dma_start(out=den_t[h:h + 1, :], in_=d1[64:65, :])
                    if koff == 0:
                        nc.vector.tensor_copy(attnTb_t[krt][0:64, :],
                                              psa[0:64, :])
                    else:
                        t64 = tmp64_p.tile([64, 512], BF16, name="t64")
                        nc.vector.tensor_copy(t64[:], psa[0:64, :])
                        nc.gpsimd.dma_start(out=attnTb_t[krt][64:128, :],
                                            in_=t64[:])
                yield

            rec_t = recip_p.tile([8, 512], F32, name="rec")
            nc.vector.reciprocal(rec_t[:], den_t[:])
            rec_d = dram.tile([8, 512], F32, tag="recip_d", name="rec_d")
            nc.sync.dma_start(out=rec_d[:], in_=rec_t[:])
            for h in range(HPC):
                krt, koff = h // 2, 64 * (h % 2)
                rb = recipb_p.tile([128, 512], F32, name="rb")
                nc.sync.dma_start(
                    out=rb[:], in_=rec_d[h:h + 1, :].partition_broadcast(128))
                nc.vector.tensor_mul(attnTb_t[krt][koff:koff + 64, :],
                                     attnTb_t[krt][koff:koff + 64, :],
                                     rb[koff:koff + 64, :])

            # ---- attention projection + AR1 ----
            ar1_in = dram.tile([D, 512], F32, tag="ar1_in", name="ar1_in")
            ar1_out = dram.tile([D, 512], F32, tag="ar1_out", name="ar1_out")
            for cc in range(2):
                wpt = []
                for r in range(4):
                    w = wproj_p.tile([128, 512], BF16, name="w_pr")
                    nc.sync.dma_start(out=w[:], in_=wproj[4 * cc + r])
                    wpt.append(w)
                for ct in range(4):
                    dct = 4 * cc + ct
                    ps = psM.tile([128, 512], F32, tag="mm", name="ps_pr")
                    for r in range(4):
                        nc.tensor.matmul(
                            ps[:], wpt[r][:, 128 * ct:128 * (ct + 1)],
                            attnTb_t[r][:], start=(r == 0), stop=(r == 3))
                    ai = arin_p.tile([128, 512], F32, name="ai")
                    nc.vector.tensor_copy(ai[:], ps[:])
                    nc.sync.dma_start(
                        out=ar1_in[:].rearrange("(i p) q -> p i q", p=128)
                        [:, dct, :], in_=ai[:])
                yield
            nc.gpsimd.collective_compute(
                "AllReduce", OP.add, replica_groups=PAIRS,
                ins=[ar1_in[:].opt()], outs=[ar1_out[:].opt()])
            state[("ar1", c)] = ar1_out

        # ============ stage 2: t1 + LN1 + fc + gelu + cproj + AR2 ===========
        def s2(c):
            ar1_out = state.pop(("ar1", c))
            art_t = []
            for i in range(DT):
                t1 = art_p.tile([128, 512], F32, name="t1")
                nc.sync.dma_start(
                    out=t1[:], in_=ar1_out[:]
                    .rearrange("(i p) q -> p i q", p=128)[:, i, :])
                xf2 = xf2_p.tile([128, 512], F32, name="xf2")
                nc.sync.dma_start(out=xf2[:], in_=xq[DT * c + i])
                nc.vector.tensor_add(t1[:], t1[:], xf2[:])
                if use_projb:
                    nc.vector.tensor_scalar_add(t1[:], t1[:],
                                                projb_sb[:, i:i + 1])
                art_t.append(t1)
            yield

            nTb_t = [None] * DT
            nT_d = dram.tile([DT, 128, 512], F32, tag="nT_d", name="nT_d")

            def mk_n(i, urstd_b, g_sb, b_sb, use_gb):
                nf = nf_p.tile([128, 512], F32, name="nf")
                nc.vector.tensor_sub(nf[:], art_t[i][:], urstd_b[:])
                if use_gb:
                    nc.vector.tensor_scalar(nf[:], nf[:], g_sb[:, i:i + 1],
                                            b_sb[:, i:i + 1], OP.mult, OP.add)
                nb = nTb_p.tile([128, 512], BF16, tag="nTb", name="nb")
                nc.vector.tensor_copy(nb[:], nf[:])
                nc.gpsimd.dma_start(out=nT_d[i], in_=nf[:])
                nTb_t[i] = nb

            layernorm(art_t, mk_n, g1_sb, b1_sb, use_g1b1)
            state[("nT_d", c)] = nT_d
            yield

            # ---- fc + gelu ----
            gT_t = []
            for fg in range(4):
                wft = []
                for d in range(DT):
                    w = wfc_p.tile([128, 512], BF16, name="w_fc")
                    nc.sync.dma_start(out=w[:], in_=wfc[8 * fg + d])
                    wft.append(w)
                for fi in range(4):
                    f = 4 * fg + fi
                    ps = psM.tile([128, 512], F32, tag="mm", name="ps_fc")
                    for d in range(DT):
                        nc.tensor.matmul(
                            ps[:], wft[d][:, 128 * fi:128 * (fi + 1)],
                            nTb_t[d][:], start=(d == 0), stop=(d == DT - 1))
                    gt = gT_p.tile([128, 512], BF16, name="gt")
                    nc.scalar.activation(gt[:], ps[:], AF.Gelu_apprx_tanh,
                                         bias=fcb_sb[:, f:f + 1])
                    gT_t.append(gt)
                yield

            # ---- cproj + AR2 ----
            ar2_in = dram.tile([D, 512], F32, tag="ar2_in", name="ar2_in")
            ar2_out = dram.tile([D, 512], F32, tag="ar2_out", name="ar2_out")
            for p2 in range(4):
                wct = []
                for f in range(16):
                    w = wcproj_p.tile([128, 256], BF16, name="w_cp")
                    nc.sync.dma_start(out=w[:], in_=wcproj[16 * p2 + f])
                    wct.append(w)
                for ci in range(2):
                    dct = 2 * p2 + ci
                    ps = psM.tile([128, 512], F32, tag="mm", name="ps_cp")
                    for f in range(16):
                        nc.tensor.matmul(
                            ps[:], wct[f][:, 128 * ci:128 * (ci + 1)],
                            gT_t[f][:], start=(f == 0), stop=(f == 15))
                    mt = mT_p.tile([128, 512], F32, name="mt")
                    nc.vector.tensor_copy(mt[:], ps[:])
                    nc.sync.dma_start(
                        out=ar2_in[:].rearrange("(i p) q -> p i q", p=128)
                        [:, dct, :], in_=mt[:])
                yield
            nc.gpsimd.collective_compute(
                "AllReduce", OP.add, replica_groups=PAIRS,
                ins=[ar2_in[:].opt()], outs=[ar2_out[:].opt()])
            state[("ar2", c)] = ar2_out

        # ================= stage 3: t2 + LN2 + store ========================
        def s3(c):
            ar2_out = state.pop(("ar2", c))
            nT_d = state.pop(("nT_d", c))
            mar_t = []
            for i in range(DT):
                m2 = mar_p.tile([128, 512], F32, name="m2")
                nc.sync.dma_start(
                    out=m2[:], in_=ar2_out[:]
                    .rearrange("(i p) q -> p i q", p=128)[:, i, :])
                nf2 = nf2_p.tile([128, 512], F32, name="nf2")
                nc.sync.dma_start(out=nf2[:], in_=nT_d[i])
                nc.vector.tensor_add(m2[:], m2[:], nf2[:])
                if use_cprojb:
                    nc.vector.tensor_scalar_add(m2[:], m2[:],
                                                cprojb_sb[:, i:i + 1])
                mar_t.append(m2)
                yield

            def mk_h(i, urstd_b, g_sb, b_sb, use_gb):
                ht = hT_p.tile([128, 512], F32, tag="hT", name="ht")
                nc.vector.tensor_sub(ht[:], mar_t[i][:], urstd_b[:])
                if use_gb:
                    nc.vector.tensor_scalar(ht[:], ht[:], g_sb[:, i:i + 1],
                                            b_sb[:, i:i + 1], OP.mult, OP.add)
                nc.gpsimd.dma_start(out=out[DT * c + i], in_=ht[:])

            layernorm(mar_t, mk_h, g2_sb, b2_sb, use_g2b2)
            yield

        # pipelined emission: deep skew + fine-grained interleave so the
        # PE stream alternates attention (ACT-paced) with MLP matmuls
        def run(g):
            for _ in g:
                pass

        def il(*gs):
            gs = list(gs)
            while gs:
                for g in list(gs):
                    try:
                        next(g)
                    except StopIteration:
                        gs.remove(g)

        run(s1(0))
        run(s1(1))
        il(s1(2), s2(0))
        il(s1(3), s2(1))
        il(s2(2), s3(0))
        il(s2(3), s3(1))
        run(s3(2))
        run(s3(3))

    nc.compile()
    return nc


_cache = {}


def _get_program(flags):
    if flags not in _cache:
        _cache[flags] = _build(*flags)
    return _cache[flags]


def _pack(a, rows, cols):
    """[R, C] -> [R//rows * C//cols, rows, cols], row-tile-major."""
    R, C = a.shape
    return np.ascontiguousarray(
        a.reshape(R // rows, rows, C // cols, cols).transpose(0, 2, 1, 3)
        .reshape(-1, rows, cols))


def _prepare_inputs(inputs):
    x = np.asarray(inputs["x"], np.float32)
    weight = float(np.asarray(inputs["weight"]).reshape(-1)[0])
    linear_w = np.asarray(inputs["linear_w"], np.float32)
    linear_b = np.asarray(inputs["linear_b"], np.float32)
    proj_w = np.asarray(inputs["proj_w"], np.float32)
    proj_b = np.asarray(inputs["proj_b"], np.float32)
    ln1_g = np.asarray(inputs["ln1_g"], np.float32)
    ln1_b = np.asarray(inputs["ln1_b"], np.float32)
    fc_w = np.asarray(inputs["fc_w"], np.float32)
    fc_b = np.asarray(inputs["fc_b"], np.float32)
    cproj_w = np.asarray(inputs["cproj_w"], np.float32)
    cproj_b = np.asarray(inputs["cproj_b"], np.float32)
    ln2_g = np.asarray(inputs["ln2_g"], np.float32)
    ln2_b = np.asarray(inputs["ln2_b"], np.float32)
    idx = np.asarray(inputs["idx"]).astype(np.int64).reshape(-1)
    forcing = bool(np.asarray(inputs["attn_forcing"]).reshape(-1)[0])

    flags = (
        bool(linear_b[:2048].any()),      # use_bqk
        bool(linear_b[2048:].any()),      # use_bv
        bool(proj_b.any()),
        bool(cproj_b.any()),
        bool((ln1_g != 1).any() or ln1_b.any()),
        bool((ln2_g != 1).any() or ln2_b.any()),
    )

    if forcing:
        lnw = float(np.log(weight)) if weight >= 1e-37 else -1e9
        pos = np.arange(S)
        lna_all = np.where(pos[None, :] >= idx[:, None], lnw, 0.0) \
            .astype(np.float32)
    else:
        lna_all = np.zeros((B, S), np.float32)

    tri_np = np.triu(np.ones((128, 128), np.float32)).astype(BF)

    in_maps = []
    for core in range(N_CORES):
        b, r = core // 2, core % 2
        q_cols = slice(512 * r, 512 * (r + 1))
        k_cols = slice(1024 + 512 * r, 1024 + 512 * (r + 1))
        v_cols = slice(2048 + 512 * r, 2048 + 512 * (r + 1))
        xT = np.ascontiguousarray(x[b].T)                       # [D, S]
        wqk_full = np.concatenate([linear_w[:, q_cols], linear_w[:, k_cols]],
                                  axis=1)                       # [D, 1024]
        # _pack gives (row-tile, col-tile) order; kernel indexes are
        # (col-chunk, row-tile) for wqk/wproj/wfc, (col-pass, row-tile)
        # for wcproj, (chunk, row-tile) for xq -- so swap the axes.
        def swap(p, n_rt, n_ct):
            t = p.reshape(n_rt, n_ct, p.shape[1], p.shape[2])
            return np.ascontiguousarray(
                t.transpose(1, 0, 2, 3).reshape(-1, p.shape[1], p.shape[2]))

        in_maps.append({
            "xq": swap(_pack(xT, 128, 512), DT, NCH),            # (c, d)
            "wqk": swap(_pack(wqk_full.astype(BF), 128, 512), 8, 2),  # (cc,d)
            "bqk": np.ascontiguousarray(
                np.concatenate([linear_b[q_cols], linear_b[k_cols]])),
            "wv": _pack(linear_w[:, v_cols].astype(BF), 128, 512),    # (d)
            "bv": np.ascontiguousarray(linear_b[v_cols]).astype(BF),
            "wproj": swap(_pack(proj_w[512 * r:512 * (r + 1), :].astype(BF),
                                128, 512), 4, 2),                # (cc, r)
            "projb": proj_b,
            "wfc": swap(_pack(fc_w[:, 2048 * r:2048 * (r + 1)].astype(BF),
                              128, 512), 8, 4),                  # (fg, d)
            "fcb": np.ascontiguousarray(fc_b[2048 * r:2048 * (r + 1)]),
            "wcproj": swap(_pack(cproj_w[2048 * r:2048 * (r + 1), :]
                                 .astype(BF), 128, 256), 16, 4),  # (p2, f)
            "cprojb": cproj_b,
            "g1": ln1_g, "b1": ln1_b, "g2": ln2_g, "b2": ln2_b,
            "lna": lna_all[b],
            "tri": tri_np,
        })
    return flags, in_maps


def _unpack_out(o):
    """[NCH*DT, 128, 512] tiles (c, i) -> [S, D] token-major."""
    hT = o.reshape(NCH, DT, 128, 512).transpose(1, 2, 0, 3).reshape(D, S)
    return hT.T


def _run(inputs, trace=False):
    flags, in_maps = _prepare_inputs(inputs)
    nc = _get_program(flags)
    r = run_bass_kernel_spmd(nc, in_maps, core_ids=list(range(N_CORES)),
                             trace=trace)
    outs = np.stack(
        [np.ascontiguousarray(_unpack_out(r.results[2 * b]["out"]))
         for b in range(B)], axis=0).astype(np.float32)
    return outs, r


def kernel(**inputs) -> np.ndarray:
    outs, _ = _run(inputs, trace=False)
    return outs



# revision 13
# speedup vs baseline: 1.2715x; 1.2715x over previous
"""Dense transformer block on 8 TRN2 NeuronCores.

Sharding: data-parallel over batch (4 pairs). Within each pair:
  - Attention is Megatron head-parallel (8 heads per core, all tokens).
  - The post-attention half (residual+LN1+MLP+LN2) is chunk-parallel:
    after the attention projection, partial sums for two 512-token chunks
    are combined with ONE pairwise ReduceScatter arranged so each core
    receives whole reduced chunks (core r owns chunks {r, 2+r}).  The MLP
    then runs full-width locally (fc [1024,4096], cproj [4096,1024]) so
    there is no second collective at all.

Device layout is feature-major: activations live as [d_model, tokens].
Attention: scores computed transposed, softmax without max-subtraction,
attention-forcing folded into the exp bias, denominator rides the attn@V
matmul as a 65th ones-column of V; the per-head 1/den is broadcast to the
head-pair partition ranges with a small PE matmul (selector @ recip-rows)
instead of a DRAM round-trip.  LayerNorm stats are partition-dim sums via
PE with bf16 rhs; the rstd / mean*rstd rows are broadcast to 128
partitions with a PE matmul (ones-row @ stat-row).  The bf16 fc-input
tiles double as the s3 residual (no n stash to DRAM).

Emission: A(c) = attention for chunk c (all 4 chunks), B(k) = MLP for the
k-th owned chunk.  Order: A0 A1 [RS0] A2 il(A3+[RS1], B0.head) B0.rest
il(B0-tail, B1.head) B1.rest — the PE always has independent work while
collectives and LN stat round-trips are in flight, and ACT table switches
(exp / sqrt / gelu) are kept to a few per window.
"""

import numpy as np
import ml_dtypes

import concourse.bacc as bacc
import concourse.mybir as mybir
import concourse.tile as tile
from concourse.bass_utils import run_bass_kernel_spmd

F32 = mybir.dt.float32
BF16 = mybir.dt.bfloat16
AF = mybir.ActivationFunctionType
OP = mybir.AluOpType

B, S, D, H, HD, FF = 4, 2048, 1024, 16, 64, 4096
N_CORES = 8
PAIRS = [[0, 1], [2, 3], [4, 5], [6, 7]]
CH = 512                 # tokens per chunk
NCH = S // CH            # 4
DT = D // 128            # 8 d-tiles
FT = FF // 128           # 32 f-tiles
KT = S // 128            # 16 kpos tiles
EPS = 1e-5
BF = ml_dtypes.bfloat16


def _build(use_bqk, use_bv, use_projb, use_cprojb, use_g1b1, use_g2b2):
    nc = bacc.Bacc("TRN2", target_bir_lowering=False, debug=False,
                   enable_asserts=True, num_devices=N_CORES)

    # ---- DRAM inputs (tile-packed on host) ----
    xqb = nc.dram_tensor("xqb", [NCH, 2, 128, 4 * 512], BF16,
                         kind="ExternalInput")          # bf16 x^T (c, half)
    xo = nc.dram_tensor("xo", [2 * DT, 128, 512], F32,
                        kind="ExternalInput")           # f32 x^T own chunks
    wqk = nc.dram_tensor("wqk", [16, 128, 512], BF16, kind="ExternalInput")
    bqk = nc.dram_tensor("bqk", [1024], F32, kind="ExternalInput")
    wv = nc.dram_tensor("wv", [8, 128, 512], BF16, kind="ExternalInput")
    bv = nc.dram_tensor("bv", [512], BF16, kind="ExternalInput")
    wproj = nc.dram_tensor("wproj", [8, 128, 512], BF16, kind="ExternalInput")
    projb = nc.dram_tensor("projb", [D], F32, kind="ExternalInput")
    wfc = nc.dram_tensor("wfc", [8, 2, 128, 4 * 512], BF16,
                         kind="ExternalInput")          # (fg, half) x (d,q)
    fcb = nc.dram_tensor("fcb", [FF], F32, kind="ExternalInput")
    wcp = nc.dram_tensor("wcp", [4, 4, 128, 8 * 256], BF16,
                         kind="ExternalInput")          # (p4, qtr) x (f,q)
    cprojb = nc.dram_tensor("cprojb", [D], F32, kind="ExternalInput")
    g1 = nc.dram_tensor("g1", [D], F32, kind="ExternalInput")
    b1 = nc.dram_tensor("b1", [D], F32, kind="ExternalInput")
    g2 = nc.dram_tensor("g2", [D], F32, kind="ExternalInput")
    b2 = nc.dram_tensor("b2", [D], F32, kind="ExternalInput")
    lna = nc.dram_tensor("lna", [S], F32, kind="ExternalInput")
    tri = nc.dram_tensor("tri", [128, 128], BF16, kind="ExternalInput")
    sel = nc.dram_tensor("sel", [4, 8, 128], F32, kind="ExternalInput")
    # output: own chunks (k, i) tiles; host reassembles
    out = nc.dram_tensor("out", [2 * DT, 128, 512], F32,
                         kind="ExternalOutput")

    from contextlib import ExitStack
    with tile.TileContext(nc) as tc, ExitStack() as ctx:
        def pool(name, bufs, space="SBUF"):
            return ctx.enter_context(
                tc.tile_pool(name=name, bufs=bufs, space=space))

        const = pool("const", 1)
        wres = pool("wres", 1)          # resident attention weights
        xb_p = pool("xb_p", 2)          # bf16 x half-chunks [128, 2048]
        qTb_p = pool("qTb_p", 4)
        pP = pool("pP", 3)
        attnTb_p = pool("attnTb_p", 4)
        den_p = pool("den_p", 1)
        den1_p = pool("den1_p", 2)
        rec_p = pool("rec_p", 1)
        tmp64_p = pool("tmp64_p", 2)
        ai_p = pool("ai_p", 2)          # proj partial f32 tiles
        t1_p = pool("t1_p", 10)         # B: residual tiles f32 (t1 AND n+m)
        xf2_p = pool("xf2_p", 2)
        cast_p = pool("cast_p", 2)      # LN bf16 casts
        sq_p = pool("sq_p", 2)
        strow_p = pool("strow_p", 1)
        nTb_p = pool("nTb_p", 8)        # bf16 n tiles (fc rhs + s3 residual)
        tmpn_p = pool("tmpn_p", 1)
        wf_p = pool("wf_p", 3)          # fc weight half-groups [128, 2048]
        gT_p = pool("gT_p", 32)         # gelu outputs bf16
        wc_p = pool("wc_p", 3)          # cproj weight quarter [128, 2048]
        hT_p = pool("hT_p", 2)
        psS = pool("psS", 2, "PSUM")
        psA = pool("psA", 2, "PSUM")
        psM = pool("psM", 2, "PSUM")
        psL = pool("psL", 2, "PSUM")
        dram = pool("dram", 2, "DRAM")

        # ---- constants ----
        kt_sb = const.tile([128, 4 * S], BF16, name="kt_sb")
        kt_v = kt_sb[:].rearrange("p (r q) -> p r q", q=S)
        v_sb = const.tile([128, KT * 520], BF16, name="v_sb")
        v_v = v_sb[:].rearrange("p (t e) -> p t e", e=520)

        tri_sb = const.tile([128, 128], BF16, name="tri_sb")
        nc.sync.dma_start(out=tri_sb[:], in_=tri[:])
        lna_sb = const.tile([128, KT], F32, name="lna_sb")
        nc.sync.dma_start(out=lna_sb[:],
                          in_=lna.rearrange("(t p) -> p t", p=128))
        ones_col_b = const.tile([128, 1], BF16, name="ones_col_b")
        nc.vector.memset(ones_col_b[:], 1.0)
        ones_row_f = const.tile([1, 128], F32, name="ones_row_f")
        nc.vector.memset(ones_row_f[:], 1.0)
        # per-krt head-pair selectors (host-built): sel[krt][2krt, 0:64]=1,
        # sel[krt][2krt+1, 64:128]=1 -> rb = sel^T @ rec broadcasts head
        # 2krt over partitions 0..63 and 2krt+1 over 64..127.
        sel_t = []
        for krt in range(4):
            s = const.tile([8, 128], F32, name=f"sel{krt}")
            nc.sync.dma_start(out=s[:], in_=sel[krt])
            sel_t.append(s)
        eps_sb = const.tile([1, 1], F32, name="eps_sb")
        nc.vector.memset(eps_sb[:], EPS)
        fcb_sb = const.tile([128, FT], F32, name="fcb_sb")
        nc.sync.dma_start(out=fcb_sb[:],
                          in_=fcb.rearrange("(i p) -> p i", p=128))

        def vec8(name, t):
            sb = const.tile([128, DT], F32, name=name)
            nc.sync.dma_start(out=sb[:],
                              in_=t.rearrange("(i p) -> p i", p=128))
            return sb

        bqk_sb = vec8("bqk_sb", bqk) if use_bqk else None
        projb_sb = vec8("projb_sb", projb) if use_projb else None
        cprojb_sb = vec8("cprojb_sb", cprojb) if use_cprojb else None
        g1_sb = vec8("g1_sb", g1) if use_g1b1 else None
        b1_sb = vec8("b1_sb", b1) if use_g1b1 else None
        g2_sb = vec8("g2_sb", g2) if use_g2b2 else None
        b2_sb = vec8("b2_sb", b2) if use_g2b2 else None
        if use_bv:
            ones_row_b = const.tile([1, 128], BF16, name="ones_row_b")
            nc.vector.memset(ones_row_b[:], 1.0)
            bv_sb = const.tile([1, 512], BF16, name="bv_sb")
            nc.sync.dma_start(out=bv_sb[:],
                              in_=bv.rearrange("(o q) -> o q", o=1))

        # ---- resident attention weights (one batched DMA each) ----
        wqk_sb = wres.tile([128, 16 * 512], BF16, name="wqk_sb")
        nc.sync.dma_start(
            out=wqk_sb[:].rearrange("p (i q) -> p i q", q=512),
            in_=wqk.rearrange("i p q -> p i q"))
        wqk_t = [wqk_sb[:, 512 * i:512 * (i + 1)] for i in range(16)]
        wv_sb = wres.tile([128, 8 * 512], BF16, name="wv_sb")
        nc.sync.dma_start(
            out=wv_sb[:].rearrange("p (i q) -> p i q", q=512),
            in_=wv.rearrange("i p q -> p i q"))
        wv_t = [wv_sb[:, 512 * i:512 * (i + 1)] for i in range(8)]
        wpr_sb = wres.tile([128, 8 * 512], BF16, name="wpr_sb")
        nc.sync.dma_start(
            out=wpr_sb[:].rearrange("p (i q) -> p i q", q=512),
            in_=wproj.rearrange("i p q -> p i q"))
        wpr_t = [wpr_sb[:, 512 * i:512 * (i + 1)] for i in range(8)]

        # ---- ReduceScatter buffers ----
        rs_in = [dram.tile([2 * D, 512], F32, tag=f"rsi{j}",
                           name=f"rs_in{j}") for j in range(2)]
        rs_out = [dram.tile([D, 512], F32, tag=f"rso{j}",
                            name=f"rs_out{j}") for j in range(2)]

        # ---- shared LN helpers ----
        def ln_stats(src_t):
            """src_t: 8 f32 [128,512] tiles -> (rstd_ps, urstd_ps) PSUM
            broadcast tiles [128,512]."""
            ps_sumA = psL.tile([1, 512], F32, tag="psl", name="ps_sumA")
            ps_sumB = psL.tile([1, 512], F32, tag="psl", name="ps_sumB")
            for i in range(DT):
                tb = cast_p.tile([128, 512], BF16, name="tb")
                nc.vector.tensor_copy(tb[:], src_t[i][:])
                nc.tensor.matmul(ps_sumA[:], ones_col_b[:], tb[:],
                                 start=(i == 0), stop=(i == DT - 1))
                sqt = sq_p.tile([128, 512], BF16, name="sqt")
                nc.scalar.activation(sqt[:], src_t[i][:], AF.Square)
                nc.tensor.matmul(ps_sumB[:], ones_col_b[:], sqt[:],
                                 start=(i == 0), stop=(i == DT - 1))
            st = strow_p.tile([1, 3 * 512], F32, tag="st", name="st")
            sA, sB2, sC = st[:, 0:512], st[:, 512:1024], st[:, 1024:1536]
            nc.scalar.activation(sA, ps_sumA[:], AF.Copy, scale=1.0 / D)  # u
            nc.scalar.activation(sB2, ps_sumB[:], AF.Identity,
                                 bias=eps_sb[:], scale=1.0 / D)   # msq+eps
            nc.scalar.activation(sC, sA, AF.Square)               # u^2
            nc.vector.tensor_sub(sB2, sB2, sC)                    # var
            nc.vector.reciprocal(sC, sB2)                         # 1/var
            nc.scalar.activation(sB2, sC, AF.Sqrt)                # rstd
            nc.vector.tensor_mul(sC, sA, sB2)                     # u*rstd
            rstd_ps = psL.tile([128, 512], F32, tag="psl", name="rstd_ps")
            nc.tensor.matmul(rstd_ps[:], ones_row_f[:], sB2,
                             start=True, stop=True)
            urstd_ps = psL.tile([128, 512], F32, tag="psl", name="urstd_ps")
            nc.tensor.matmul(urstd_ps[:], ones_row_f[:], sC,
                             start=True, stop=True)
            return rstd_ps, urstd_ps

        def layernorm_to_bf16(src_t, g_sb, b_sb, use_gb):
            rstd_ps, urstd_ps = ln_stats(src_t)
            out_t = []
            for i in range(DT):
                tmpn = tmpn_p.tile([128, 512], F32, name="tmpn")
                nc.vector.tensor_mul(tmpn[:], src_t[i][:], rstd_ps[:])
                nb = nTb_p.tile([128, 512], BF16, tag="nTb", name="nb")
                nc.vector.tensor_sub(nb[:], tmpn[:], urstd_ps[:])
                if use_gb:
                    nc.vector.tensor_scalar(nb[:], nb[:], g_sb[:, i:i + 1],
                                            b_sb[:, i:i + 1], OP.mult, OP.add)
                out_t.append(nb)
            return out_t

        # ================= A: attention for chunk c ========================
        def A(c):
            tok = slice(CH * c, CH * (c + 1))
            xh = []
            for half in range(2):
                t = xb_p.tile([128, 4 * 512], BF16, name="xh")
                nc.sync.dma_start(out=t[:], in_=xqb[c, half])
                xh.append(t)
            xTb_t = [xh[d // 4][:, 512 * (d % 4):512 * (d % 4 + 1)]
                     for d in range(DT)]

            qTb_t = []
            for cc in range(2):
                for ct in range(4):
                    i = 4 * cc + ct
                    ps = psM.tile([128, 512], F32, tag="mm", name="ps_qk")
                    for d in range(DT):
                        nc.tensor.matmul(
                            ps[:],
                            wqk_t[8 * cc + d][:, 128 * ct:128 * (ct + 1)],
                            xTb_t[d], start=(d == 0), stop=(d == DT - 1))
                    if i < 4:
                        dest_t = qTb_p.tile([128, 512], BF16, name="qTb")
                        qTb_t.append(dest_t)
                        dest = dest_t[:]
                    else:
                        dest = kt_v[:, i - 4, tok]
                    if use_bqk:
                        nc.scalar.activation(dest, ps[:], AF.Identity,
                                             bias=bqk_sb[:, i:i + 1])
                    else:
                        nc.vector.tensor_copy(dest, ps[:])
                yield

            for tt in range(4):
                tg = 4 * c + tt
                ps = psM.tile([128, 512], F32, tag="mm", name="ps_v")
                for d in range(DT):
                    nc.tensor.matmul(
                        ps[:], xTb_t[d][:, 128 * tt:128 * (tt + 1)],
                        wv_t[d], start=(d == 0),
                        stop=(d == DT - 1 and not use_bv))
                if use_bv:
                    nc.tensor.matmul(ps[:], ones_row_b[:], bv_sb[:],
                                     start=False, stop=True)
                v3 = v_v[:, tg, :].rearrange("p (h e) -> p h e", e=65)
                nc.vector.tensor_copy(v3[:, :, 0:64],
                                      ps[:].rearrange("p (h e) -> p h e",
                                                      e=64))
                nc.vector.memset(v3[:, :, 64:65], 1.0)
            yield

            # ---- attention (head pairs on distinct row groups) ----
            attnTb_t = [attnTb_p.tile([128, 512], BF16, tag="attnTb",
                                      name=f"attnTb{r}") for r in range(4)]
            den_t = den_p.tile([8, 512], F32, name="den")
            nt = 4 * (c + 1)
            for krt in range(4):
                h0, h1 = 2 * krt, 2 * krt + 1
                q0 = qTb_t[krt][0:64, :]
                q1 = qTb_t[krt][64:128, :]
                psa0 = psA.tile([65, 512], F32, tag="psa", name="psa0")
                psa1 = psA.tile([65, 512], F32, tag="psa", name="psa1")
                for t in range(nt):
                    j = t - 4 * c
                    qo = 128 * j if j >= 0 else 0
                    ks = kt_v[:, krt, 128 * t:128 * (t + 1)]
                    ps0 = psS.tile([128, 512], F32, tag="ps_s", name="ps0")
                    ps1 = psS.tile([128, 512], F32, tag="ps_s", name="ps1")
                    nc.tensor.matmul(ps0[:, qo:], ks[0:64, :], q0[:, qo:],
                                     start=True, stop=True)
                    nc.tensor.matmul(ps1[:, qo:], ks[64:128, :], q1[:, qo:],
                                     start=True, stop=True)
                    pt0 = pP.tile([128, 512], BF16, tag="pt", name="pt0")
                    pt1 = pP.tile([128, 512], BF16, tag="pt", name="pt1")
                    nc.scalar.activation(pt0[:, qo:], ps0[:, qo:], AF.Exp,
                                         bias=lna_sb[:, t:t + 1], scale=0.125)
                    nc.scalar.activation(pt1[:, qo:], ps1[:, qo:], AF.Exp,
                                         bias=lna_sb[:, t:t + 1], scale=0.125)
                    if j >= 0:
                        nc.vector.tensor_mul(pt0[:, qo:qo + 128],
                                             pt0[:, qo:qo + 128], tri_sb[:])
                        nc.vector.tensor_mul(pt1[:, qo:qo + 128],
                                             pt1[:, qo:qo + 128], tri_sb[:])
                    nc.tensor.matmul(psa0[:, qo:],
                                     v_v[:, t, 65 * h0:65 * (h0 + 1)],
                                     pt0[:, qo:], start=(t == 0),
                                     stop=(t == nt - 1))
                    nc.tensor.matmul(psa1[:, qo:],
                                     v_v[:, t, 65 * h1:65 * (h1 + 1)],
                                     pt1[:, qo:], start=(t == 0),
                                     stop=(t == nt - 1))
                for h, psa, koff in ((h0, psa0, 0), (h1, psa1, 64)):
                    d1 = den1_p.tile([65, 512], F32, tag="d1", name="d1")
                    nc.vector.tensor_copy(d1[64:65, :], psa[64:65, :])
                    nc.scalar.dma_start(out=den_t[h:h + 1, :],
                                        in_=d1[64:65, :])
                    if koff == 0:
                        nc.vector.tensor_copy(attnTb_t[krt][0:64, :],
                                              psa[0:64, :])
                    else:
                        t64 = tmp64_p.tile([64, 512], BF16, name="t64")
                        nc.vector.tensor_copy(t64[:], psa[0:64, :])
                        nc.scalar.dma_start(out=attnTb_t[krt][64:128, :],
                                            in_=t64[:])
                yield

            rec_t = rec_p.tile([8, 512], F32, name="rec")
            nc.vector.reciprocal(rec_t[:], den_t[:])
            for krt in range(4):
                rb = psL.tile([128, 512], F32, tag="psl", name="rb")
                nc.tensor.matmul(rb[:], sel_t[krt][:], rec_t[:],
                                 start=True, stop=True)
                nc.vector.tensor_mul(attnTb_t[krt][:], attnTb_t[krt][:],
                                     rb[:])
            yield

            # ---- attention projection -> rs_in block ----
            blk = c % 2
            ri = rs_in[c // 2][:] \
                .rearrange("(k i p) q -> k i p q", k=2, p=128)
            for cc in range(2):
                for ct in range(4):
                    dct = 4 * cc + ct
                    ps = psM.tile([128, 512], F32, tag="mm", name="ps_pr")
                    for r in range(4):
                        nc.tensor.matmul(
                            ps[:],
                            wpr_t[4 * cc + r][:, 128 * ct:128 * (ct + 1)],
                            attnTb_t[r][:], start=(r == 0), stop=(r == 3))
                    ai = ai_p.tile([128, 512], F32, name="ai")
                    nc.vector.tensor_copy(ai[:], ps[:])
                    nc.gpsimd.dma_start(out=ri[blk, dct], in_=ai[:])
                yield

        # ============== B: full-width MLP for owned chunk k ================
        def B(k):
            ro = rs_out[k][:].rearrange("(i p) q -> i p q", p=128)
            t1_t = []
            for i in range(DT):
                t1 = t1_p.tile([128, 512], F32, name="t1")
                nc.sync.dma_start(out=t1[:], in_=ro[i])
                xf2 = xf2_p.tile([128, 512], F32, name="xf2")
                nc.sync.dma_start(out=xf2[:], in_=xo[DT * k + i])
                nc.vector.tensor_add(t1[:], t1[:], xf2[:])
                if use_projb:
                    nc.vector.tensor_scalar_add(t1[:], t1[:],
                                                projb_sb[:, i:i + 1])
                t1_t.append(t1)
            yield

            nTb_t = layernorm_to_bf16(t1_t, g1_sb, b1_sb, use_g1b1)
            yield

            # ---- fc + gelu ----
            gT_t = []
            for fg in range(8):
                wfh = []
                for half in range(2):
                    t = wf_p.tile([128, 4 * 512], BF16, name="wfh")
                    nc.sync.dma_start(out=t[:], in_=wfc[fg, half])
                    wfh.append(t)
                for ct in range(4):
                    f = 4 * fg + ct
                    ps = psM.tile([128, 512], F32, tag="mm", name="ps_fc")
                    for d in range(DT):
                        w = wfh[d // 4]
                        dd = d % 4
                        nc.tensor.matmul(
                            ps[:],
                            w[:, 512 * dd + 128 * ct:512 * dd + 128 * (ct + 1)],
                            nTb_t[d][:], start=(d == 0), stop=(d == DT - 1))
                    gt = gT_p.tile([128, 512], BF16, name="gt")
                    nc.scalar.activation(gt[:], ps[:], AF.Gelu_apprx_tanh,
                                         bias=fcb_sb[:, f:f + 1])
                    gT_t.append(gt)
                yield

            # ---- cproj (full width; contraction over all 32 f-tiles) ----
            mar_t = []
            for p4 in range(4):
                wcq = []
                for qtr in range(4):
                    t = wc_p.tile([128, 8 * 256], BF16, name="wcq")
                    nc.sync.dma_start(out=t[:], in_=wcp[p4, qtr])
                    wcq.append(t)
                for ci in range(2):
                    dct = 2 * p4 + ci
                    ps = psM.tile([128, 512], F32, tag="mm", name="ps_cp")
                    for f in range(FT):
                        w = wcq[f // 8]
                        fi = f % 8
                        nc.tensor.matmul(
                            ps[:],
                            w[:, 256 * fi + 128 * ci:256 * fi + 128 * (ci + 1)],
                            gT_t[f][:], start=(f == 0), stop=(f == FT - 1))
                    m2 = t1_p.tile([128, 512], F32, name="t1")
                    nc.vector.tensor_add(m2[:], ps[:], nTb_t[dct][:])
                    if use_cprojb:
                        nc.vector.tensor_scalar_add(
                            m2[:], m2[:], cprojb_sb[:, dct:dct + 1])
                    mar_t.append(m2)
                yield

            # ---- LN2 -> output ----
            rstd_ps, urstd_ps = ln_stats(mar_t)
            for i in range(DT):
                ht = hT_p.tile([128, 512], F32, tag="hT", name="ht")
                nc.vector.tensor_mul(ht[:], mar_t[i][:], rstd_ps[:])
                nc.vector.tensor_sub(ht[:], ht[:], urstd_ps[:])
                if use_g2b2:
                    nc.vector.tensor_scalar(ht[:], ht[:], g2_sb[:, i:i + 1],
                                            b2_sb[:, i:i + 1],
                                            OP.mult, OP.add)
                nc.scalar.dma_start(out=out[DT * k + i], in_=ht[:])
            yield

        # ---- emission ----
        def run(g):
            for _ in g:
                pass

        def il(ga, gb, gb_limit=None):
            """Round-robin ga/gb; advance gb at most gb_limit steps, then
            finish ga.  Returns gb (possibly unfinished)."""
            steps = 0
            done_a = done_b = False
            while not (done_a and done_b):
                if not done_a:
                    try:
                        next(ga)
                    except StopIteration:
                        done_a = True
                if not done_b:
                    if gb_limit is not None and steps >= gb_limit:
                        done_b = True
                    else:
                        try:
                            next(gb)
                            steps += 1
                        except StopIteration:
                            done_b = True
            return gb

        def trigger_rs(j):
            nc.gpsimd.collective_compute(
                "ReduceScatter", OP.add, replica_groups=PAIRS,
                ins=[rs_in[j][:].opt()], outs=[rs_out[j][:].opt()])

        def A3_then_rs():
            yield from A(3)
            trigger_rs(1)

        import os
        sched = os.environ.get("K_SCHED", "overlap")
        run(A(0))
        run(A(1))
        trigger_rs(0)
        run(A(2))
        if sched == "serial":
            run(A3_then_rs())
            run(B(0))
            run(B(1))
        else:
            # overlap A3 with B0's residual+LN1 (2 steps), then B0's MLP
            # with B1's residual+LN1 (2 steps) pulled in near the end.
            b0 = B(0)
            il(A3_then_rs(), b0, gb_limit=2)
            b1 = B(1)
            il(b0, b1, gb_limit=2)
            run(b1)

    nc.compile()
    return nc


_cache = {}


def _get_program(flags):
    if flags not in _cache:
        _cache[flags] = _build(*flags)
    return _cache[flags]


def _pack(a, rows, cols):
    """[R, C] -> [R//rows * C//cols, rows, cols], row-tile-major."""
    R, C = a.shape
    return np.ascontiguousarray(
        a.reshape(R // rows, rows, C // cols, cols).transpose(0, 2, 1, 3)
        .reshape(-1, rows, cols))


def _swap(p, n_rt, n_ct):
    """_pack gives (row-tile, col-tile) order; swap to (col, row)."""
    t = p.reshape(n_rt, n_ct, p.shape[1], p.shape[2])
    return np.ascontiguousarray(
        t.transpose(1, 0, 2, 3).reshape(-1, p.shape[1], p.shape[2]))


def _prepare_inputs(inputs):
    x = np.asarray(inputs["x"], np.float32)
    weight = float(np.asarray(inputs["weight"]).reshape(-1)[0])
    linear_w = np.asarray(inputs["linear_w"], np.float32)
    linear_b = np.asarray(inputs["linear_b"], np.float32)
    proj_w = np.asarray(inputs["proj_w"], np.float32)
    proj_b = np.asarray(inputs["proj_b"], np.float32)
    ln1_g = np.asarray(inputs["ln1_g"], np.float32)
    ln1_b = np.asarray(inputs["ln1_b"], np.float32)
    fc_w = np.asarray(inputs["fc_w"], np.float32)
    fc_b = np.asarray(inputs["fc_b"], np.float32)
    cproj_w = np.asarray(inputs["cproj_w"], np.float32)
    cproj_b = np.asarray(inputs["cproj_b"], np.float32)
    ln2_g = np.asarray(inputs["ln2_g"], np.float32)
    ln2_b = np.asarray(inputs["ln2_b"], np.float32)
    idx = np.asarray(inputs["idx"]).astype(np.int64).reshape(-1)
    forcing = bool(np.asarray(inputs["attn_forcing"]).reshape(-1)[0])

    flags = (
        bool(linear_b[:2048].any()),      # use_bqk
        bool(linear_b[2048:].any()),      # use_bv
        bool(proj_b.any()),
        bool(cproj_b.any()),
        bool((ln1_g != 1).any() or ln1_b.any()),
        bool((ln2_g != 1).any() or ln2_b.any()),
    )

    if forcing:
        lnw = float(np.log(weight)) if weight >= 1e-37 else -1e9
        pos = np.arange(S)
        lna_all = np.where(pos[None, :] >= idx[:, None], lnw, 0.0) \
            .astype(np.float32)
    else:
        lna_all = np.zeros((B, S), np.float32)

    tri_np = np.triu(np.ones((128, 128), np.float32)).astype(BF)
    sel_np = np.zeros((4, 8, 128), np.float32)
    for krt in range(4):
        sel_np[krt, 2 * krt, 0:64] = 1.0
        sel_np[krt, 2 * krt + 1, 64:128] = 1.0

    # ---- global (all-core) MLP weights ----
    # wfc tile (fg, d) of [128,512]; regroup free dim as (d, q) halves
    wfc_p = _swap(_pack(fc_w.astype(BF), 128, 512), DT, 8)   # (fg, d)
    wfc_g = np.ascontiguousarray(
        wfc_p.reshape(8, 2, 4, 128, 512).transpose(0, 1, 3, 2, 4)
        .reshape(8, 2, 128, 4 * 512))
    # wcp tile (p4, f) of [128,256]; quarters of 8 f-tiles
    wcp_p = _swap(_pack(cproj_w.astype(BF), 128, 256), FT, 4)  # (p4, f)
    wcp_g = np.ascontiguousarray(
        wcp_p.reshape(4, 4, 8, 128, 256).transpose(0, 1, 3, 2, 4)
        .reshape(4, 4, 128, 8 * 256))

    in_maps = []
    for core in range(N_CORES):
        b, r = core // 2, core % 2
        q_cols = slice(512 * r, 512 * (r + 1))
        k_cols = slice(1024 + 512 * r, 1024 + 512 * (r + 1))
        v_cols = slice(2048 + 512 * r, 2048 + 512 * (r + 1))
        xT = np.ascontiguousarray(x[b].T)                       # [D, S]
        wqk_full = np.concatenate([linear_w[:, q_cols], linear_w[:, k_cols]],
                                  axis=1)                       # [D, 1024]

        xq_t = _pack(xT, 128, 512)                  # (d, c): index d*NCH+c
        xq_dc = xq_t.reshape(DT, NCH, 128, 512)
        # xqb[c, half] = [128, (d%4, q)] bf16
        xqb = np.ascontiguousarray(
            xq_dc.transpose(1, 0, 2, 3).reshape(NCH, 2, 4, 128, 512)
            .transpose(0, 1, 3, 2, 4).reshape(NCH, 2, 128, 4 * 512)
        ).astype(BF)
        own = [r, 2 + r]
        xo_np = np.ascontiguousarray(
            xq_dc[:, own].transpose(1, 0, 2, 3).reshape(2 * DT, 128, 512))

        in_maps.append({
            "xqb": xqb,
            "xo": xo_np,
            "wqk": _swap(_pack(wqk_full.astype(BF), 128, 512), 8, 2),
            "bqk": np.ascontiguousarray(
                np.concatenate([linear_b[q_cols], linear_b[k_cols]])),
            "wv": _pack(linear_w[:, v_cols].astype(BF), 128, 512),
            "bv": np.ascontiguousarray(linear_b[v_cols]).astype(BF),
            "wproj": _swap(_pack(proj_w[512 * r:512 * (r + 1), :].astype(BF),
                                 128, 512), 4, 2),
            "projb": proj_b,
            "wfc": wfc_g, "fcb": fc_b,
            "wcp": wcp_g, "cprojb": cproj_b,
            "g1": ln1_g, "b1": ln1_b, "g2": ln2_g, "b2": ln2_b,
            "lna": lna_all[b],
            "tri": tri_np,
            "sel": sel_np,
        })
    return flags, in_maps


def _unpack_out(o_pair):
    """o_pair: [out_core0, out_core1] each [2*DT, 128, 512] -> [S, D]."""
    hT = np.empty((D, S), np.float32)
    for r in range(2):
        o = o_pair[r].reshape(2, DT, 128, 512)
        for k, c in enumerate([r, 2 + r]):
            hT[:, CH * c:CH * (c + 1)] = o[k].reshape(D, 512)
    return hT.T


def _run(inputs, trace=False):
    flags, in_maps = _prepare_inputs(inputs)
    nc = _get_program(flags)
    r = run_bass_kernel_spmd(nc, in_maps, core_ids=list(range(N_CORES)),
                             trace=trace)
    outs = np.stack(
        [_unpack_out([r.results[2 * b]["out"], r.results[2 * b + 1]["out"]])
         for b in range(B)], axis=0).astype(np.float32)
    return outs, r


def kernel(**inputs) -> np.ndarray:
    outs, _ = _run(inputs, trace=False)
    return outs


# revision 22
# speedup vs baseline: 1.3698x; 1.0773x over previous
"""Dense transformer block on 8 TRN2 NeuronCores.

Sharding: data-parallel over batch (4 pairs). Within each pair:
  - Attention is Megatron head-parallel (8 heads per core, all tokens).
  - The post-attention half (residual+LN1+MLP+LN2) is chunk-parallel:
    after the attention projection, partial sums for two 512-token chunks
    are combined with ONE pairwise ReduceScatter arranged so each core
    receives whole reduced chunks (core r owns chunks {r, 2+r}).  The MLP
    then runs full-width locally (fc [1024,4096], cproj [4096,1024]) so
    there is no second collective at all.

Device layout is feature-major: activations live as [d_model, tokens].
Attention: scores computed transposed, softmax without max-subtraction,
attention-forcing folded into the exp bias, denominator rides the attn@V
matmul as a 65th ones-column of V; the per-head 1/den is broadcast to the
head-pair partition ranges with a small PE matmul (selector @ recip-rows)
instead of a DRAM round-trip.  LayerNorm stats are partition-dim sums via
PE with bf16 rhs; the rstd / mean*rstd rows are broadcast to 128
partitions with a PE matmul (ones-row @ stat-row).  The bf16 fc-input
tiles double as the s3 residual (no n stash to DRAM).

Emission: A(c) = attention for chunk c (all 4 chunks), B(k) = MLP for the
k-th owned chunk.  Order: A0 A1 [RS0] A2 il(A3+[RS1], B0.head) B0.rest
il(B0-tail, B1.head) B1.rest — the PE always has independent work while
collectives and LN stat round-trips are in flight, and ACT table switches
(exp / sqrt / gelu) are kept to a few per window.
"""

import numpy as np
import ml_dtypes

import concourse.bacc as bacc
import concourse.mybir as mybir
import concourse.tile as tile
from concourse.bass_utils import run_bass_kernel_spmd

F32 = mybir.dt.float32
BF16 = mybir.dt.bfloat16
AF = mybir.ActivationFunctionType
OP = mybir.AluOpType

B, S, D, H, HD, FF = 4, 2048, 1024, 16, 64, 4096
N_CORES = 8
PAIRS = [[0, 1], [2, 3], [4, 5], [6, 7]]
CH = 512                 # tokens per chunk
NCH = S // CH            # 4
DT = D // 128            # 8 d-tiles
FT = FF // 128           # 32 f-tiles
KT = S // 128            # 16 kpos tiles
EPS = 1e-5
BF = ml_dtypes.bfloat16


def _build(use_bqk, use_bv, use_projb, use_cprojb, use_g1b1, use_g2b2):
    nc = bacc.Bacc("TRN2", target_bir_lowering=False, debug=False,
                   enable_asserts=True, num_devices=N_CORES)

    # ---- DRAM inputs (tile-packed on host) ----
    xqb = nc.dram_tensor("xqb", [NCH, 2, 128, 4 * 512], BF16,
                         kind="ExternalInput")          # bf16 x^T (c, half)
    xo = nc.dram_tensor("xo", [2 * DT, 128, 512], F32,
                        kind="ExternalInput")           # f32 x^T own chunks
    wqk = nc.dram_tensor("wqk", [16, 128, 512], BF16, kind="ExternalInput")
    bqk = nc.dram_tensor("bqk", [1024], F32, kind="ExternalInput")
    wv = nc.dram_tensor("wv", [8, 128, 512], BF16, kind="ExternalInput")
    bv = nc.dram_tensor("bv", [512], BF16, kind="ExternalInput")
    wproj = nc.dram_tensor("wproj", [8, 128, 512], BF16, kind="ExternalInput")
    projb = nc.dram_tensor("projb", [D], F32, kind="ExternalInput")
    wfc = nc.dram_tensor("wfc", [8, 2, 128, 4 * 512], BF16,
                         kind="ExternalInput")          # (fg, half) x (d,q)
    fcb = nc.dram_tensor("fcb", [FF], F32, kind="ExternalInput")
    wcp = nc.dram_tensor("wcp", [4, 4, 128, 8 * 256], BF16,
                         kind="ExternalInput")          # (p4, qtr) x (f,q)
    cprojb = nc.dram_tensor("cprojb", [D], F32, kind="ExternalInput")
    g1 = nc.dram_tensor("g1", [D], F32, kind="ExternalInput")
    b1 = nc.dram_tensor("b1", [D], F32, kind="ExternalInput")
    g2 = nc.dram_tensor("g2", [D], F32, kind="ExternalInput")
    b2 = nc.dram_tensor("b2", [D], F32, kind="ExternalInput")
    lna = nc.dram_tensor("lna", [S], F32, kind="ExternalInput")
    tri = nc.dram_tensor("tri", [128, 128], BF16, kind="ExternalInput")
    sel = nc.dram_tensor("sel", [4, 8, 128], F32, kind="ExternalInput")
    # output: own chunks (k, i) tiles; host reassembles
    out = nc.dram_tensor("out", [2 * DT, 128, 512], F32,
                         kind="ExternalOutput")

    from contextlib import ExitStack
    with tile.TileContext(nc) as tc, ExitStack() as ctx:
        def pool(name, bufs, space="SBUF"):
            return ctx.enter_context(
                tc.tile_pool(name=name, bufs=bufs, space=space))

        const = pool("const", 1)
        wres = pool("wres", 1)          # resident attention weights
        xb_p = pool("xb_p", 2)          # bf16 x half-chunks [128, 2048]
        qTb_p = pool("qTb_p", 4)
        pP = pool("pP", 3)
        attnTb_p = pool("attnTb_p", 4)
        den_p = pool("den_p", 1)
        den1_p = pool("den1_p", 2)
        rec_p = pool("rec_p", 1)
        tmp64_p = pool("tmp64_p", 2)
        ai_p = pool("ai_p", 2)          # proj partial f32 tiles
        t1_p = pool("t1_p", 10)         # B: residual tiles f32 (t1 AND n+m)
        xf2_p = pool("xf2_p", 2)
        cast_p = pool("cast_p", 2)      # LN bf16 casts
        sq_p = pool("sq_p", 2)
        strow_p = pool("strow_p", 1)
        nTb_p = pool("nTb_p", 8)        # bf16 n tiles (fc rhs + s3 residual)
        tmpn_p = pool("tmpn_p", 1)
        wf_p = pool("wf_p", 3)          # fc weight half-groups [128, 2048]
        gT_p = pool("gT_p", 32)         # gelu outputs bf16
        wc_p = pool("wc_p", 3)          # cproj weight quarter [128, 2048]
        hT_p = pool("hT_p", 2)
        psS = pool("psS", 2, "PSUM")
        psA = pool("psA", 2, "PSUM")
        psM = pool("psM", 2, "PSUM")
        psL = pool("psL", 2, "PSUM")
        dram = pool("dram", 2, "DRAM")

        # ---- constants ----
        kt_sb = const.tile([128, 4 * S], BF16, name="kt_sb")
        kt_v = kt_sb[:].rearrange("p (r q) -> p r q", q=S)
        v_sb = const.tile([128, KT * 520], BF16, name="v_sb")
        v_v = v_sb[:].rearrange("p (t e) -> p t e", e=520)

        tri_sb = const.tile([128, 128], BF16, name="tri_sb")
        nc.sync.dma_start(out=tri_sb[:], in_=tri[:])
        lna_sb = const.tile([128, KT], F32, name="lna_sb")
        nc.sync.dma_start(out=lna_sb[:],
                          in_=lna.rearrange("(t p) -> p t", p=128))
        ones_col_b = const.tile([128, 1], BF16, name="ones_col_b")
        nc.vector.memset(ones_col_b[:], 1.0)
        ones_row_f = const.tile([1, 128], F32, name="ones_row_f")
        nc.vector.memset(ones_row_f[:], 1.0)
        # per-krt head-pair selectors (host-built): sel[krt][2krt, 0:64]=1,
        # sel[krt][2krt+1, 64:128]=1 -> rb = sel^T @ rec broadcasts head
        # 2krt over partitions 0..63 and 2krt+1 over 64..127.
        sel_t = []
        for krt in range(4):
            s = const.tile([8, 128], F32, name=f"sel{krt}")
            nc.sync.dma_start(out=s[:], in_=sel[krt])
            sel_t.append(s)
        eps_sb = const.tile([1, 1], F32, name="eps_sb")
        nc.vector.memset(eps_sb[:], EPS)
        fcb_sb = const.tile([128, FT], F32, name="fcb_sb")
        nc.sync.dma_start(out=fcb_sb[:],
                          in_=fcb.rearrange("(i p) -> p i", p=128))

        def vec8(name, t):
            sb = const.tile([128, DT], F32, name=name)
            nc.sync.dma_start(out=sb[:],
                              in_=t.rearrange("(i p) -> p i", p=128))
            return sb

        bqk_sb = vec8("bqk_sb", bqk) if use_bqk else None
        projb_sb = vec8("projb_sb", projb) if use_projb else None
        cprojb_sb = vec8("cprojb_sb", cprojb) if use_cprojb else None
        g1_sb = vec8("g1_sb", g1) if use_g1b1 else None
        b1_sb = vec8("b1_sb", b1) if use_g1b1 else None
        g2_sb = vec8("g2_sb", g2) if use_g2b2 else None
        b2_sb = vec8("b2_sb", b2) if use_g2b2 else None
        if use_bv:
            ones_row_b = const.tile([1, 128], BF16, name="ones_row_b")
            nc.vector.memset(ones_row_b[:], 1.0)
            bv_sb = const.tile([1, 512], BF16, name="bv_sb")
            nc.sync.dma_start(out=bv_sb[:],
                              in_=bv.rearrange("(o q) -> o q", o=1))

        def load_xh(c):
            xh = []
            for half in range(2):
                t = xb_p.tile([128, 4 * 512], BF16, name="xh")
                nc.sync.dma_start(out=t[:], in_=xqb[c, half])
                xh.append(t)
            return xh

        # prefetch chunk 0's x before the big weight DMAs so the first
        # QKV matmuls start as early as possible
        xh0 = load_xh(0)

        # ---- resident attention weights (one batched DMA each) ----
        wqk_sb = wres.tile([128, 16 * 512], BF16, name="wqk_sb")
        nc.sync.dma_start(
            out=wqk_sb[:].rearrange("p (i q) -> p i q", q=512),
            in_=wqk.rearrange("i p q -> p i q"))
        wqk_t = [wqk_sb[:, 512 * i:512 * (i + 1)] for i in range(16)]
        wv_sb = wres.tile([128, 8 * 512], BF16, name="wv_sb")
        nc.sync.dma_start(
            out=wv_sb[:].rearrange("p (i q) -> p i q", q=512),
            in_=wv.rearrange("i p q -> p i q"))
        wv_t = [wv_sb[:, 512 * i:512 * (i + 1)] for i in range(8)]
        wpr_sb = wres.tile([128, 8 * 512], BF16, name="wpr_sb")
        nc.sync.dma_start(
            out=wpr_sb[:].rearrange("p (i q) -> p i q", q=512),
            in_=wproj.rearrange("i p q -> p i q"))
        wpr_t = [wpr_sb[:, 512 * i:512 * (i + 1)] for i in range(8)]

        # ---- ReduceScatter buffers ----
        rs_in = [dram.tile([2 * D, 512], F32, tag=f"rsi{j}",
                           name=f"rs_in{j}") for j in range(2)]
        rs_out = [dram.tile([D, 512], F32, tag=f"rso{j}",
                            name=f"rs_out{j}") for j in range(2)]

        # ---- shared LN helpers ----
        def ln_stat_begin(stat_pool, stat_tag):
            ps_sumA = stat_pool.tile([1, 512], F32, tag=stat_tag,
                                     name="ps_sumA")
            ps_sumB = stat_pool.tile([1, 512], F32, tag=stat_tag,
                                     name="ps_sumB")
            return ps_sumA, ps_sumB

        def ln_stat_tile(ps_sumA, ps_sumB, src, i):
            tb = cast_p.tile([128, 512], BF16, name="tb")
            nc.vector.tensor_copy(tb[:], src[:])
            nc.tensor.matmul(ps_sumA[:], ones_col_b[:], tb[:],
                             start=(i == 0), stop=(i == DT - 1))
            sqt = sq_p.tile([128, 512], BF16, name="sqt")
            nc.scalar.activation(sqt[:], src[:], AF.Square)
            nc.tensor.matmul(ps_sumB[:], ones_col_b[:], sqt[:],
                             start=(i == 0), stop=(i == DT - 1))

        def ln_stat_finish(ps_sumA, ps_sumB, bc_pool, bc_tag):
            st = strow_p.tile([1, 3 * 512], F32, tag="st", name="st")
            sA, sB2, sC = st[:, 0:512], st[:, 512:1024], st[:, 1024:1536]
            nc.scalar.activation(sA, ps_sumA[:], AF.Copy, scale=1.0 / D)  # u
            nc.scalar.activation(sB2, ps_sumB[:], AF.Identity,
                                 bias=eps_sb[:], scale=1.0 / D)   # msq+eps
            nc.scalar.activation(sC, sA, AF.Square)               # u^2
            nc.vector.tensor_sub(sB2, sB2, sC)                    # var
            nc.vector.reciprocal_approx_fast(sC, sB2)             # 1/var
            nc.scalar.activation(sB2, sC, AF.Sqrt)                # rstd
            nc.vector.tensor_mul(sC, sA, sB2)                     # u*rstd
            rstd_ps = bc_pool.tile([128, 512], F32, tag=bc_tag,
                                   name="rstd_ps")
            nc.tensor.matmul(rstd_ps[:], ones_row_f[:], sB2,
                             start=True, stop=True)
            urstd_ps = bc_pool.tile([128, 512], F32, tag=bc_tag,
                                    name="urstd_ps")
            nc.tensor.matmul(urstd_ps[:], ones_row_f[:], sC,
                             start=True, stop=True)
            return rstd_ps, urstd_ps

        def ln_stats(src_t):
            pA, pB = ln_stat_begin(psL, "psl")
            for i in range(DT):
                ln_stat_tile(pA, pB, src_t[i], i)
            return ln_stat_finish(pA, pB, psL, "psl")

        def layernorm_to_bf16(src_t, g_sb, b_sb, use_gb):
            rstd_ps, urstd_ps = ln_stats(src_t)
            out_t = []
            for i in range(DT):
                tmpn = tmpn_p.tile([128, 512], F32, name="tmpn")
                nc.vector.tensor_mul(tmpn[:], src_t[i][:], rstd_ps[:])
                nb = nTb_p.tile([128, 512], BF16, tag="nTb", name="nb")
                nc.vector.tensor_sub(nb[:], tmpn[:], urstd_ps[:])
                if use_gb:
                    nc.vector.tensor_scalar(nb[:], nb[:], g_sb[:, i:i + 1],
                                            b_sb[:, i:i + 1], OP.mult, OP.add)
                out_t.append(nb)
            return out_t

        # ================= A: attention for chunk c ========================
        def A(c, xh=None):
            tok = slice(CH * c, CH * (c + 1))
            if xh is None:
                xh = load_xh(c)
            xTb_t = [xh[d // 4][:, 512 * (d % 4):512 * (d % 4 + 1)]
                     for d in range(DT)]

            qTb_t = []
            for cc in range(2):
                for ct in range(4):
                    i = 4 * cc + ct
                    ps = psM.tile([128, 512], F32, tag="mm", name="ps_qk")
                    for d in range(DT):
                        nc.tensor.matmul(
                            ps[:],
                            wqk_t[8 * cc + d][:, 128 * ct:128 * (ct + 1)],
                            xTb_t[d], start=(d == 0), stop=(d == DT - 1))
                    if i < 4:
                        dest_t = qTb_p.tile([128, 512], BF16, name="qTb")
                        qTb_t.append(dest_t)
                        dest = dest_t[:]
                    else:
                        dest = kt_v[:, i - 4, tok]
                    if use_bqk:
                        nc.scalar.activation(dest, ps[:], AF.Identity,
                                             bias=bqk_sb[:, i:i + 1])
                    else:
                        nc.vector.tensor_copy(dest, ps[:])
                yield

            for tt in range(4):
                tg = 4 * c + tt
                ps = psM.tile([128, 512], F32, tag="mm", name="ps_v")
                for d in range(DT):
                    nc.tensor.matmul(
                        ps[:], xTb_t[d][:, 128 * tt:128 * (tt + 1)],
                        wv_t[d], start=(d == 0),
                        stop=(d == DT - 1 and not use_bv))
                if use_bv:
                    nc.tensor.matmul(ps[:], ones_row_b[:], bv_sb[:],
                                     start=False, stop=True)
                v3 = v_v[:, tg, :].rearrange("p (h e) -> p h e", e=65)
                nc.vector.tensor_copy(v3[:, :, 0:64],
                                      ps[:].rearrange("p (h e) -> p h e",
                                                      e=64))
                nc.vector.memset(v3[:, :, 64:65], 1.0)
            yield

            # ---- attention (head pairs on distinct row groups) ----
            attnTb_t = [attnTb_p.tile([128, 512], BF16, tag="attnTb",
                                      name=f"attnTb{r}") for r in range(4)]
            den_t = den_p.tile([8, 512], F32, name="den")
            nt = 4 * (c + 1)
            for krt in range(4):
                h0, h1 = 2 * krt, 2 * krt + 1
                q0 = qTb_t[krt][0:64, :]
                q1 = qTb_t[krt][64:128, :]
                psa0 = psA.tile([65, 512], F32, tag="psa", name="psa0")
                psa1 = psA.tile([65, 512], F32, tag="psa", name="psa1")
                for t in range(nt):
                    j = t - 4 * c
                    qo = 128 * j if j >= 0 else 0
                    ks = kt_v[:, krt, 128 * t:128 * (t + 1)]
                    ps0 = psS.tile([128, 512], F32, tag="ps_s", name="ps0")
                    ps1 = psS.tile([128, 512], F32, tag="ps_s", name="ps1")
                    nc.tensor.matmul(ps0[:, qo:], ks[0:64, :], q0[:, qo:],
                                     start=True, stop=True)
                    nc.tensor.matmul(ps1[:, qo:], ks[64:128, :], q1[:, qo:],
                                     start=True, stop=True)
                    pt0 = pP.tile([128, 512], BF16, tag="pt", name="pt0")
                    pt1 = pP.tile([128, 512], BF16, tag="pt", name="pt1")
                    nc.scalar.activation(pt0[:, qo:], ps0[:, qo:], AF.Exp,
                                         bias=lna_sb[:, t:t + 1], scale=0.125)
                    nc.scalar.activation(pt1[:, qo:], ps1[:, qo:], AF.Exp,
                                         bias=lna_sb[:, t:t + 1], scale=0.125)
                    if j >= 0:
                        nc.vector.tensor_mul(pt0[:, qo:qo + 128],
                                             pt0[:, qo:qo + 128], tri_sb[:])
                        nc.vector.tensor_mul(pt1[:, qo:qo + 128],
                                             pt1[:, qo:qo + 128], tri_sb[:])
                    nc.tensor.matmul(psa0[:, qo:],
                                     v_v[:, t, 65 * h0:65 * (h0 + 1)],
                                     pt0[:, qo:], start=(t == 0),
                                     stop=(t == nt - 1))
                    nc.tensor.matmul(psa1[:, qo:],
                                     v_v[:, t, 65 * h1:65 * (h1 + 1)],
                                     pt1[:, qo:], start=(t == 0),
                                     stop=(t == nt - 1))
                for h, psa, koff in ((h0, psa0, 0), (h1, psa1, 64)):
                    d1 = den1_p.tile([65, 512], F32, tag="d1", name="d1")
                    nc.vector.tensor_copy(d1[64:65, :], psa[64:65, :])
                    nc.scalar.dma_start(out=den_t[h:h + 1, :],
                                        in_=d1[64:65, :])
                    if koff == 0:
                        nc.vector.tensor_copy(attnTb_t[krt][0:64, :],
                                              psa[0:64, :])
                    else:
                        t64 = tmp64_p.tile([64, 512], BF16, name="t64")
                        nc.vector.tensor_copy(t64[:], psa[0:64, :])
                        nc.scalar.dma_start(out=attnTb_t[krt][64:128, :],
                                            in_=t64[:])
                yield

            rec_t = rec_p.tile([8, 512], F32, name="rec")
            nc.vector.reciprocal_approx_fast(rec_t[:], den_t[:])
            for krt in range(4):
                rb = psL.tile([128, 512], F32, tag="psl", name="rb")
                nc.tensor.matmul(rb[:], sel_t[krt][:], rec_t[:],
                                 start=True, stop=True)
                nc.vector.tensor_mul(attnTb_t[krt][:], attnTb_t[krt][:],
                                     rb[:])
            yield

            # ---- attention projection -> rs_in block ----
            blk = c % 2
            ri = rs_in[c // 2][:] \
                .rearrange("(k i p) q -> k i p q", k=2, p=128)
            for cc in range(2):
                for ct in range(4):
                    dct = 4 * cc + ct
                    ps = psM.tile([128, 512], F32, tag="mm", name="ps_pr")
                    for r in range(4):
                        nc.tensor.matmul(
                            ps[:],
                            wpr_t[4 * cc + r][:, 128 * ct:128 * (ct + 1)],
                            attnTb_t[r][:], start=(r == 0), stop=(r == 3))
                    ai = ai_p.tile([128, 512], F32, name="ai")
                    nc.vector.tensor_copy(ai[:], ps[:])
                    nc.sync.dma_start(out=ri[blk, dct], in_=ai[:])
                yield

        # ============== B: full-width MLP for owned chunk k ================
        def B(k):
            ro = rs_out[k][:].rearrange("(i p) q -> i p q", p=128)
            t1_t = []
            for i in range(DT):
                t1 = t1_p.tile([128, 512], F32, name="t1")
                nc.sync.dma_start(out=t1[:], in_=ro[i])
                xf2 = xf2_p.tile([128, 512], F32, name="xf2")
                nc.sync.dma_start(out=xf2[:], in_=xo[DT * k + i])
                nc.vector.tensor_add(t1[:], t1[:], xf2[:])
                if use_projb:
                    nc.vector.tensor_scalar_add(t1[:], t1[:],
                                                projb_sb[:, i:i + 1])
                t1_t.append(t1)
            yield

            nTb_t = layernorm_to_bf16(t1_t, g1_sb, b1_sb, use_g1b1)
            yield

            # ---- fc + gelu ----
            gT_t = []
            for fg in range(8):
                wfh = []
                for half in range(2):
                    t = wf_p.tile([128, 4 * 512], BF16, name="wfh")
                    nc.sync.dma_start(out=t[:], in_=wfc[fg, half])
                    wfh.append(t)
                for ct in range(4):
                    f = 4 * fg + ct
                    ps = psM.tile([128, 512], F32, tag="mm", name="ps_fc")
                    for d in range(DT):
                        w = wfh[d // 4]
                        dd = d % 4
                        nc.tensor.matmul(
                            ps[:],
                            w[:, 512 * dd + 128 * ct:512 * dd + 128 * (ct + 1)],
                            nTb_t[d][:], start=(d == 0), stop=(d == DT - 1))
                    gt = gT_p.tile([128, 512], BF16, name="gt")
                    nc.scalar.activation(gt[:], ps[:], AF.Gelu_apprx_tanh,
                                         bias=fcb_sb[:, f:f + 1])
                    gT_t.append(gt)
                yield

            # ---- cproj (full width; contraction over all 32 f-tiles),
            # with LN2 stats emitted incrementally per output pair ----
            mar_t = []
            pA2, pB2 = ln_stat_begin(psS, "ps_s")
            for p4 in range(4):
                wcq = []
                for qtr in range(4):
                    t = wc_p.tile([128, 8 * 256], BF16, name="wcq")
                    nc.sync.dma_start(out=t[:], in_=wcp[p4, qtr])
                    wcq.append(t)
                for ci in range(2):
                    dct = 2 * p4 + ci
                    ps = psM.tile([128, 512], F32, tag="mm", name="ps_cp")
                    for f in range(FT):
                        w = wcq[f // 8]
                        fi = f % 8
                        nc.tensor.matmul(
                            ps[:],
                            w[:, 256 * fi + 128 * ci:256 * fi + 128 * (ci + 1)],
                            gT_t[f][:], start=(f == 0), stop=(f == FT - 1))
                    m2 = t1_p.tile([128, 512], F32, name="t1")
                    nc.vector.tensor_add(m2[:], ps[:], nTb_t[dct][:])
                    if use_cprojb:
                        nc.vector.tensor_scalar_add(
                            m2[:], m2[:], cprojb_sb[:, dct:dct + 1])
                    mar_t.append(m2)
                    ln_stat_tile(pA2, pB2, m2, dct)
                yield

            # ---- LN2 -> output ----
            rstd_ps, urstd_ps = ln_stat_finish(pA2, pB2, psA, "psa")
            for i in range(DT):
                ht = hT_p.tile([128, 512], F32, tag="hT", name="ht")
                nc.vector.tensor_mul(ht[:], mar_t[i][:], rstd_ps[:])
                nc.vector.tensor_sub(ht[:], ht[:], urstd_ps[:])
                if use_g2b2:
                    nc.vector.tensor_scalar(ht[:], ht[:], g2_sb[:, i:i + 1],
                                            b2_sb[:, i:i + 1],
                                            OP.mult, OP.add)
                nc.scalar.dma_start(out=out[DT * k + i], in_=ht[:])
            yield

        # ---- emission ----
        def run(g):
            for _ in g:
                pass

        def il(ga, gb, gb_limit=None):
            """Round-robin ga/gb; advance gb at most gb_limit steps, then
            finish ga.  Returns gb (possibly unfinished)."""
            steps = 0
            done_a = done_b = False
            while not (done_a and done_b):
                if not done_a:
                    try:
                        next(ga)
                    except StopIteration:
                        done_a = True
                if not done_b:
                    if gb_limit is not None and steps >= gb_limit:
                        done_b = True
                    else:
                        try:
                            next(gb)
                            steps += 1
                        except StopIteration:
                            done_b = True
            return gb

        def trigger_rs(j):
            nc.gpsimd.collective_compute(
                "ReduceScatter", OP.add, replica_groups=PAIRS,
                ins=[rs_in[j][:].opt()], outs=[rs_out[j][:].opt()])

        def A3_then_rs():
            yield from A(3)
            trigger_rs(1)

        import os
        sched = os.environ.get("K_SCHED", "full")
        run(A(0, xh0))
        run(A(1))
        trigger_rs(0)
        run(A(2))
        if sched == "serial":
            run(A3_then_rs())
            run(B(0))
            run(B(1))
        elif sched == "o1":
            b0 = B(0)
            il(A3_then_rs(), b0, gb_limit=2)
            run(b0)
            run(B(1))
        else:
            # overlap A3 with B0's residual+LN1 (2 steps), then B0's MLP
            # with B1's residual+LN1 (2 steps) pulled in near the end.
            b0 = B(0)
            il(A3_then_rs(), b0, gb_limit=2)
            b1 = B(1)
            il(b0, b1, gb_limit=2)
            run(b1)

    nc.compile()
    return nc


_cache = {}


def _get_program(flags):
    if flags not in _cache:
        _cache[flags] = _build(*flags)
    return _cache[flags]


def _pack(a, rows, cols):
    """[R, C] -> [R//rows * C//cols, rows, cols], row-tile-major."""
    R, C = a.shape
    return np.ascontiguousarray(
        a.reshape(R // rows, rows, C // cols, cols).transpose(0, 2, 1, 3)
        .reshape(-1, rows, cols))


def _swap(p, n_rt, n_ct):
    """_pack gives (row-tile, col-tile) order; swap to (col, row)."""
    t = p.reshape(n_rt, n_ct, p.shape[1], p.shape[2])
    return np.ascontiguousarray(
        t.transpose(1, 0, 2, 3).reshape(-1, p.shape[1], p.shape[2]))


def _prepare_inputs(inputs):
    x = np.asarray(inputs["x"], np.float32)
    weight = float(np.asarray(inputs["weight"]).reshape(-1)[0])
    linear_w = np.asarray(inputs["linear_w"], np.float32)
    linear_b = np.asarray(inputs["linear_b"], np.float32)
    proj_w = np.asarray(inputs["proj_w"], np.float32)
    proj_b = np.asarray(inputs["proj_b"], np.float32)
    ln1_g = np.asarray(inputs["ln1_g"], np.float32)
    ln1_b = np.asarray(inputs["ln1_b"], np.float32)
    fc_w = np.asarray(inputs["fc_w"], np.float32)
    fc_b = np.asarray(inputs["fc_b"], np.float32)
    cproj_w = np.asarray(inputs["cproj_w"], np.float32)
    cproj_b = np.asarray(inputs["cproj_b"], np.float32)
    ln2_g = np.asarray(inputs["ln2_g"], np.float32)
    ln2_b = np.asarray(inputs["ln2_b"], np.float32)
    idx = np.asarray(inputs["idx"]).astype(np.int64).reshape(-1)
    forcing = bool(np.asarray(inputs["attn_forcing"]).reshape(-1)[0])

    flags = (
        bool(linear_b[:2048].any()),      # use_bqk
        bool(linear_b[2048:].any()),      # use_bv
        bool(proj_b.any()),
        bool(cproj_b.any()),
        bool((ln1_g != 1).any() or ln1_b.any()),
        bool((ln2_g != 1).any() or ln2_b.any()),
    )

    if forcing:
        lnw = float(np.log(weight)) if weight >= 1e-37 else -1e9
        pos = np.arange(S)
        lna_all = np.where(pos[None, :] >= idx[:, None], lnw, 0.0) \
            .astype(np.float32)
    else:
        lna_all = np.zeros((B, S), np.float32)

    tri_np = np.triu(np.ones((128, 128), np.float32)).astype(BF)
    sel_np = np.zeros((4, 8, 128), np.float32)
    for krt in range(4):
        sel_np[krt, 2 * krt, 0:64] = 1.0
        sel_np[krt, 2 * krt + 1, 64:128] = 1.0

    # ---- global (all-core) MLP weights ----
    # wfc tile (fg, d) of [128,512]; regroup free dim as (d, q) halves
    wfc_p = _swap(_pack(fc_w.astype(BF), 128, 512), DT, 8)   # (fg, d)
    wfc_g = np.ascontiguousarray(
        wfc_p.reshape(8, 2, 4, 128, 512).transpose(0, 1, 3, 2, 4)
        .reshape(8, 2, 128, 4 * 512))
    # wcp tile (p4, f) of [128,256]; quarters of 8 f-tiles
    wcp_p = _swap(_pack(cproj_w.astype(BF), 128, 256), FT, 4)  # (p4, f)
    wcp_g = np.ascontiguousarray(
        wcp_p.reshape(4, 4, 8, 128, 256).transpose(0, 1, 3, 2, 4)
        .reshape(4, 4, 128, 8 * 256))

    in_maps = []
    for core in range(N_CORES):
        b, r = core // 2, core % 2
        q_cols = slice(512 * r, 512 * (r + 1))
        k_cols = slice(1024 + 512 * r, 1024 + 512 * (r + 1))
        v_cols = slice(2048 + 512 * r, 2048 + 512 * (r + 1))
        xT = np.ascontiguousarray(x[b].T)                       # [D, S]
        wqk_full = np.concatenate([linear_w[:, q_cols], linear_w[:, k_cols]],
                                  axis=1)                       # [D, 1024]

        xq_t = _pack(xT, 128, 512)                  # (d, c): index d*NCH+c
        xq_dc = xq_t.reshape(DT, NCH, 128, 512)
        # xqb[c, half] = [128, (d%4, q)] bf16
        xqb = np.ascontiguousarray(
            xq_dc.transpose(1, 0, 2, 3).reshape(NCH, 2, 4, 128, 512)
            .transpose(0, 1, 3, 2, 4).reshape(NCH, 2, 128, 4 * 512)
        ).astype(BF)
        own = [r, 2 + r]
        xo_np = np.ascontiguousarray(
            xq_dc[:, own].transpose(1, 0, 2, 3).reshape(2 * DT, 128, 512))

        in_maps.append({
            "xqb": xqb,
            "xo": xo_np,
            "wqk": _swap(_pack(wqk_full.astype(BF), 128, 512), 8, 2),
            "bqk": np.ascontiguousarray(
                np.concatenate([linear_b[q_cols], linear_b[k_cols]])),
            "wv": _pack(linear_w[:, v_cols].astype(BF), 128, 512),
            "bv": np.ascontiguousarray(linear_b[v_cols]).astype(BF),
            "wproj": _swap(_pack(proj_w[512 * r:512 * (r + 1), :].astype(BF),
                                 128, 512), 4, 2),
            "projb": proj_b,
            "wfc": wfc_g, "fcb": fc_b,
            "wcp": wcp_g, "cprojb": cproj_b,
            "g1": ln1_g, "b1": ln1_b, "g2": ln2_g, "b2": ln2_b,
            "lna": lna_all[b],
            "tri": tri_np,
            "sel": sel_np,
        })
    return flags, in_maps


def _unpack_out(o_pair):
    """o_pair: [out_core0, out_core1] each [2*DT, 128, 512] -> [S, D]."""
    hT = np.empty((D, S), np.float32)
    for r in range(2):
        o = o_pair[r].reshape(2, DT, 128, 512)
        for k, c in enumerate([r, 2 + r]):
            hT[:, CH * c:CH * (c + 1)] = o[k].reshape(D, 512)
    return hT.T


def _run(inputs, trace=False):
    flags, in_maps = _prepare_inputs(inputs)
    nc = _get_program(flags)
    r = run_bass_kernel_spmd(nc, in_maps, core_ids=list(range(N_CORES)),
                             trace=trace)
    outs = np.stack(
        [_unpack_out([r.results[2 * b]["out"], r.results[2 * b + 1]["out"]])
         for b in range(B)], axis=0).astype(np.float32)
    return outs, r


def kernel(**inputs) -> np.ndarray:
    outs, _ = _run(inputs, trace=False)
    return outs


# revision 28
# speedup vs baseline: 1.4230x; 1.0388x over previous
"""Dense transformer block on 8 TRN2 NeuronCores.

Sharding: data-parallel over batch (4 pairs). Within each pair:
  - Attention is Megatron head-parallel (8 heads per core, all tokens).
  - The post-attention half (residual+LN1+MLP+LN2) is chunk-parallel:
    after the attention projection, partial sums for two 512-token chunks
    are combined with ONE pairwise ReduceScatter arranged so each core
    receives whole reduced chunks (core r owns chunks {r, 2+r}).  The MLP
    then runs full-width locally (fc [1024,4096], cproj [4096,1024]) so
    there is no second collective at all.

Device layout is feature-major: activations live as [d_model, tokens].
Attention: scores computed transposed, softmax without max-subtraction,
attention-forcing folded into the exp bias, denominator rides the attn@V
matmul as a 65th ones-column of V; the per-head 1/den is broadcast to the
head-pair partition ranges with a small PE matmul (selector @ recip-rows)
instead of a DRAM round-trip.  LayerNorm stats are partition-dim sums via
PE with bf16 rhs; the rstd / mean*rstd rows are broadcast to 128
partitions with a PE matmul (ones-row @ stat-row).  The bf16 fc-input
tiles double as the s3 residual (no n stash to DRAM).

Emission: A(c) = attention for chunk c (all 4 chunks), B(k) = MLP for the
k-th owned chunk.  Order: A0 A1 [RS0] A2 il(A3+[RS1], B0.head) B0.rest
il(B0-tail, B1.head) B1.rest — the PE always has independent work while
collectives and LN stat round-trips are in flight, and ACT table switches
(exp / sqrt / gelu) are kept to a few per window.
"""

import numpy as np
import ml_dtypes

import concourse.bacc as bacc
import concourse.mybir as mybir
import concourse.tile as tile
from concourse.bass_utils import run_bass_kernel_spmd

F32 = mybir.dt.float32
BF16 = mybir.dt.bfloat16
AF = mybir.ActivationFunctionType
OP = mybir.AluOpType

B, S, D, H, HD, FF = 4, 2048, 1024, 16, 64, 4096
N_CORES = 8
PAIRS = [[0, 1], [2, 3], [4, 5], [6, 7]]
CH = 512                 # tokens per chunk
NCH = S // CH            # 4
DT = D // 128            # 8 d-tiles
FT = FF // 128           # 32 f-tiles
KT = S // 128            # 16 kpos tiles
EPS = 1e-5
BF = ml_dtypes.bfloat16


def _build(use_bqk, use_bv, use_projb, use_cprojb, use_g1b1, use_g2b2):
    nc = bacc.Bacc("TRN2", target_bir_lowering=False, debug=False,
                   enable_asserts=True, num_devices=N_CORES)

    # ---- DRAM inputs (tile-packed on host) ----
    xqb = nc.dram_tensor("xqb", [NCH, 2, 128, 4 * 512], BF16,
                         kind="ExternalInput")          # bf16 x^T (c, half)
    xo = nc.dram_tensor("xo", [2 * DT, 128, 512], F32,
                        kind="ExternalInput")           # f32 x^T own chunks
    wqk = nc.dram_tensor("wqk", [16, 128, 512], BF16, kind="ExternalInput")
    bqk = nc.dram_tensor("bqk", [1024], F32, kind="ExternalInput")
    wv = nc.dram_tensor("wv", [8, 128, 512], BF16, kind="ExternalInput")
    bv = nc.dram_tensor("bv", [512], BF16, kind="ExternalInput")
    wproj = nc.dram_tensor("wproj", [8, 128, 512], BF16, kind="ExternalInput")
    projb = nc.dram_tensor("projb", [D], F32, kind="ExternalInput")
    wfc = nc.dram_tensor("wfc", [8, 2, 128, 4 * 512], BF16,
                         kind="ExternalInput")          # (fg, half) x (d,q)
    fcb = nc.dram_tensor("fcb", [FF], F32, kind="ExternalInput")
    wcp = nc.dram_tensor("wcp", [4, 4, 128, 8 * 256], BF16,
                         kind="ExternalInput")          # (p4, qtr) x (f,q)
    cprojb = nc.dram_tensor("cprojb", [D], F32, kind="ExternalInput")
    g1 = nc.dram_tensor("g1", [D], F32, kind="ExternalInput")
    b1 = nc.dram_tensor("b1", [D], F32, kind="ExternalInput")
    g2 = nc.dram_tensor("g2", [D], F32, kind="ExternalInput")
    b2 = nc.dram_tensor("b2", [D], F32, kind="ExternalInput")
    lna = nc.dram_tensor("lna", [S], F32, kind="ExternalInput")
    tri = nc.dram_tensor("tri", [128, 128], BF16, kind="ExternalInput")
    sel = nc.dram_tensor("sel", [4, 8, 128], F32, kind="ExternalInput")
    # output: own chunks (k, i) tiles; host reassembles
    out = nc.dram_tensor("out", [2 * DT, 128, 512], F32,
                         kind="ExternalOutput")

    from contextlib import ExitStack
    with tile.TileContext(nc) as tc, ExitStack() as ctx:
        def pool(name, bufs, space="SBUF"):
            return ctx.enter_context(
                tc.tile_pool(name=name, bufs=bufs, space=space))

        const = pool("const", 1)
        wres = pool("wres", 1)          # resident attention weights
        xb_p = pool("xb_p", 2)          # bf16 x half-chunks [128, 2048]
        qTb_p = pool("qTb_p", 4)
        pP = pool("pP", 3)
        attnTb_p = pool("attnTb_p", 4)
        den_p = pool("den_p", 1)
        den1_p = pool("den1_p", 2)
        rec_p = pool("rec_p", 1)
        tmp64_p = pool("tmp64_p", 2)
        ai_p = pool("ai_p", 2)          # proj partial bf16 tiles
        rob_p = pool("rob_p", 2)        # bf16 rs_out staging
        t1_p = pool("t1_p", 10)         # B: residual tiles f32 (t1 AND n+m)
        xf2_p = pool("xf2_p", 2)
        cast_p = pool("cast_p", 2)      # LN bf16 casts
        sq_p = pool("sq_p", 2)
        strow_p = pool("strow_p", 1)
        nTb_p = pool("nTb_p", 8)        # bf16 n tiles (fc rhs + s3 residual)
        tmpn_p = pool("tmpn_p", 1)
        wf_p = pool("wf_p", 3)          # fc weight half-groups [128, 2048]
        gT_p = pool("gT_p", 32)         # gelu outputs bf16
        wc_p = pool("wc_p", 3)          # cproj weight quarter [128, 2048]
        hT_p = pool("hT_p", 2)
        psS = pool("psS", 2, "PSUM")
        psA = pool("psA", 2, "PSUM")
        psM = pool("psM", 2, "PSUM")
        psL = pool("psL", 2, "PSUM")
        dram = pool("dram", 2, "DRAM")

        # ---- constants ----
        kt_sb = const.tile([128, 4 * S], BF16, name="kt_sb")
        kt_v = kt_sb[:].rearrange("p (r q) -> p r q", q=S)
        v_sb = const.tile([128, KT * 520], BF16, name="v_sb")
        v_v = v_sb[:].rearrange("p (t e) -> p t e", e=520)

        tri_sb = const.tile([128, 128], BF16, name="tri_sb")
        nc.sync.dma_start(out=tri_sb[:], in_=tri[:])
        lna_sb = const.tile([128, KT], F32, name="lna_sb")
        nc.sync.dma_start(out=lna_sb[:],
                          in_=lna.rearrange("(t p) -> p t", p=128))
        ones_col_b = const.tile([128, 1], BF16, name="ones_col_b")
        nc.vector.memset(ones_col_b[:], 1.0)
        ones_row_f = const.tile([1, 128], F32, name="ones_row_f")
        nc.vector.memset(ones_row_f[:], 1.0)
        # per-krt head-pair selectors (host-built): sel[krt][2krt, 0:64]=1,
        # sel[krt][2krt+1, 64:128]=1 -> rb = sel^T @ rec broadcasts head
        # 2krt over partitions 0..63 and 2krt+1 over 64..127.
        sel_t = []
        for krt in range(4):
            s = const.tile([8, 128], F32, name=f"sel{krt}")
            nc.sync.dma_start(out=s[:], in_=sel[krt])
            sel_t.append(s)
        eps_sb = const.tile([1, 1], F32, name="eps_sb")
        nc.vector.memset(eps_sb[:], EPS)
        fcb_sb = const.tile([128, FT], F32, name="fcb_sb")
        nc.sync.dma_start(out=fcb_sb[:],
                          in_=fcb.rearrange("(i p) -> p i", p=128))

        def vec8(name, t):
            sb = const.tile([128, DT], F32, name=name)
            nc.sync.dma_start(out=sb[:],
                              in_=t.rearrange("(i p) -> p i", p=128))
            return sb

        bqk_sb = vec8("bqk_sb", bqk) if use_bqk else None
        projb_sb = vec8("projb_sb", projb) if use_projb else None
        cprojb_sb = vec8("cprojb_sb", cprojb) if use_cprojb else None
        g1_sb = vec8("g1_sb", g1) if use_g1b1 else None
        b1_sb = vec8("b1_sb", b1) if use_g1b1 else None
        g2_sb = vec8("g2_sb", g2) if use_g2b2 else None
        b2_sb = vec8("b2_sb", b2) if use_g2b2 else None
        if use_bv:
            ones_row_b = const.tile([1, 128], BF16, name="ones_row_b")
            nc.vector.memset(ones_row_b[:], 1.0)
            bv_sb = const.tile([1, 512], BF16, name="bv_sb")
            nc.sync.dma_start(out=bv_sb[:],
                              in_=bv.rearrange("(o q) -> o q", o=1))

        def load_xh(c):
            xh = []
            for half in range(2):
                t = xb_p.tile([128, 4 * 512], BF16, name="xh")
                nc.sync.dma_start(out=t[:], in_=xqb[c, half])
                xh.append(t)
            return xh

        # prefetch chunk 0's x before the big weight DMAs so the first
        # QKV matmuls start as early as possible
        xh0 = load_xh(0)

        # ---- resident attention weights (one batched DMA each) ----
        wqk_sb = wres.tile([128, 16 * 512], BF16, name="wqk_sb")
        nc.sync.dma_start(
            out=wqk_sb[:].rearrange("p (i q) -> p i q", q=512),
            in_=wqk.rearrange("i p q -> p i q"))
        wqk_t = [wqk_sb[:, 512 * i:512 * (i + 1)] for i in range(16)]
        wv_sb = wres.tile([128, 8 * 512], BF16, name="wv_sb")
        nc.scalar.dma_start(
            out=wv_sb[:].rearrange("p (i q) -> p i q", q=512),
            in_=wv.rearrange("i p q -> p i q"))
        wv_t = [wv_sb[:, 512 * i:512 * (i + 1)] for i in range(8)]
        wpr_sb = wres.tile([128, 8 * 512], BF16, name="wpr_sb")
        nc.scalar.dma_start(
            out=wpr_sb[:].rearrange("p (i q) -> p i q", q=512),
            in_=wproj.rearrange("i p q -> p i q"))
        wpr_t = [wpr_sb[:, 512 * i:512 * (i + 1)] for i in range(8)]

        # ---- ReduceScatter buffers (bf16 payload halves the wire) ----
        rs_in = [dram.tile([2 * D, 512], BF16, tag=f"rsi{j}",
                           name=f"rs_in{j}") for j in range(2)]
        rs_out = [dram.tile([D, 512], BF16, tag=f"rso{j}",
                            name=f"rs_out{j}") for j in range(2)]

        # ---- shared LN helpers ----
        def ln_stat_begin(stat_pool, stat_tag):
            ps_sumA = stat_pool.tile([1, 512], F32, tag=stat_tag,
                                     name="ps_sumA")
            ps_sumB = stat_pool.tile([1, 512], F32, tag=stat_tag,
                                     name="ps_sumB")
            return ps_sumA, ps_sumB

        def ln_stat_tile(ps_sumA, ps_sumB, src, i):
            tb = cast_p.tile([128, 512], BF16, name="tb")
            nc.vector.tensor_copy(tb[:], src[:])
            nc.tensor.matmul(ps_sumA[:], ones_col_b[:], tb[:],
                             start=(i == 0), stop=(i == DT - 1))
            sqt = sq_p.tile([128, 512], BF16, name="sqt")
            nc.scalar.activation(sqt[:], src[:], AF.Square)
            nc.tensor.matmul(ps_sumB[:], ones_col_b[:], sqt[:],
                             start=(i == 0), stop=(i == DT - 1))

        def ln_stat_finish(ps_sumA, ps_sumB, bc_pool, bc_tag):
            st = strow_p.tile([1, 3 * 512], F32, tag="st", name="st")
            sA, sB2, sC = st[:, 0:512], st[:, 512:1024], st[:, 1024:1536]
            nc.scalar.activation(sA, ps_sumA[:], AF.Copy, scale=1.0 / D)  # u
            nc.scalar.activation(sB2, ps_sumB[:], AF.Identity,
                                 bias=eps_sb[:], scale=1.0 / D)   # msq+eps
            nc.scalar.activation(sC, sA, AF.Square)               # u^2
            nc.vector.tensor_sub(sB2, sB2, sC)                    # var
            nc.vector.reciprocal_approx_fast(sC, sB2)             # 1/var
            nc.scalar.activation(sB2, sC, AF.Sqrt)                # rstd
            nc.vector.tensor_mul(sC, sA, sB2)                     # u*rstd
            rstd_ps = bc_pool.tile([128, 512], F32, tag=bc_tag,
                                   name="rstd_ps")
            nc.tensor.matmul(rstd_ps[:], ones_row_f[:], sB2,
                             start=True, stop=True)
            urstd_ps = bc_pool.tile([128, 512], F32, tag=bc_tag,
                                    name="urstd_ps")
            nc.tensor.matmul(urstd_ps[:], ones_row_f[:], sC,
                             start=True, stop=True)
            return rstd_ps, urstd_ps

        def ln_stats(src_t):
            pA, pB = ln_stat_begin(psL, "psl")
            for i in range(DT):
                ln_stat_tile(pA, pB, src_t[i], i)
            return ln_stat_finish(pA, pB, psL, "psl")

        def layernorm_to_bf16(src_t, g_sb, b_sb, use_gb):
            rstd_ps, urstd_ps = ln_stats(src_t)
            out_t = []
            for i in range(DT):
                tmpn = tmpn_p.tile([128, 512], F32, name="tmpn")
                nc.vector.tensor_mul(tmpn[:], src_t[i][:], rstd_ps[:])
                nb = nTb_p.tile([128, 512], BF16, tag="nTb", name="nb")
                nc.vector.tensor_sub(nb[:], tmpn[:], urstd_ps[:])
                if use_gb:
                    nc.vector.tensor_scalar(nb[:], nb[:], g_sb[:, i:i + 1],
                                            b_sb[:, i:i + 1], OP.mult, OP.add)
                out_t.append(nb)
            return out_t

        # ================= A: attention for chunk c ========================
        def A(c, xh=None):
            tok = slice(CH * c, CH * (c + 1))
            if xh is None:
                xh = load_xh(c)
            xTb_t = [xh[d // 4][:, 512 * (d % 4):512 * (d % 4 + 1)]
                     for d in range(DT)]

            qTb_t = []
            for cc in range(2):
                for ct in range(4):
                    i = 4 * cc + ct
                    ps = psM.tile([128, 512], F32, tag="mm", name="ps_qk")
                    for d in range(DT):
                        nc.tensor.matmul(
                            ps[:],
                            wqk_t[8 * cc + d][:, 128 * ct:128 * (ct + 1)],
                            xTb_t[d], start=(d == 0), stop=(d == DT - 1))
                    if i < 4:
                        dest_t = qTb_p.tile([128, 512], BF16, name="qTb")
                        qTb_t.append(dest_t)
                        dest = dest_t[:]
                    else:
                        dest = kt_v[:, i - 4, tok]
                    if use_bqk:
                        nc.scalar.activation(dest, ps[:], AF.Identity,
                                             bias=bqk_sb[:, i:i + 1])
                    else:
                        nc.vector.tensor_copy(dest, ps[:])
                yield

            for tt in range(4):
                tg = 4 * c + tt
                ps = psM.tile([128, 512], F32, tag="mm", name="ps_v")
                for d in range(DT):
                    nc.tensor.matmul(
                        ps[:], xTb_t[d][:, 128 * tt:128 * (tt + 1)],
                        wv_t[d], start=(d == 0),
                        stop=(d == DT - 1 and not use_bv))
                if use_bv:
                    nc.tensor.matmul(ps[:], ones_row_b[:], bv_sb[:],
                                     start=False, stop=True)
                v3 = v_v[:, tg, :].rearrange("p (h e) -> p h e", e=65)
                nc.vector.tensor_copy(v3[:, :, 0:64],
                                      ps[:].rearrange("p (h e) -> p h e",
                                                      e=64))
                nc.vector.memset(v3[:, :, 64:65], 1.0)
            yield

            # ---- attention (head pairs on distinct row groups) ----
            attnTb_t = [attnTb_p.tile([128, 512], BF16, tag="attnTb",
                                      name=f"attnTb{r}") for r in range(4)]
            den_t = den_p.tile([8, 512], F32, name="den")
            nt = 4 * (c + 1)
            for krt in range(4):
                h0, h1 = 2 * krt, 2 * krt + 1
                q0 = qTb_t[krt][0:64, :]
                q1 = qTb_t[krt][64:128, :]
                psa0 = psA.tile([65, 512], F32, tag="psa", name="psa0")
                psa1 = psA.tile([65, 512], F32, tag="psa", name="psa1")
                for t in range(nt):
                    j = t - 4 * c
                    qo = 128 * j if j >= 0 else 0
                    ks = kt_v[:, krt, 128 * t:128 * (t + 1)]
                    ps0 = psS.tile([128, 512], F32, tag="ps_s", name="ps0")
                    ps1 = psS.tile([128, 512], F32, tag="ps_s", name="ps1")
                    nc.tensor.matmul(ps0[:, qo:], ks[0:64, :], q0[:, qo:],
                                     start=True, stop=True)
                    nc.tensor.matmul(ps1[:, qo:], ks[64:128, :], q1[:, qo:],
                                     start=True, stop=True)
                    pt0 = pP.tile([128, 512], BF16, tag="pt", name="pt0")
                    pt1 = pP.tile([128, 512], BF16, tag="pt", name="pt1")
                    nc.scalar.activation(pt0[:, qo:], ps0[:, qo:], AF.Exp,
                                         bias=lna_sb[:, t:t + 1], scale=0.125)
                    nc.scalar.activation(pt1[:, qo:], ps1[:, qo:], AF.Exp,
                                         bias=lna_sb[:, t:t + 1], scale=0.125)
                    if j >= 0:
                        nc.vector.tensor_mul(pt0[:, qo:qo + 128],
                                             pt0[:, qo:qo + 128], tri_sb[:])
                        nc.vector.tensor_mul(pt1[:, qo:qo + 128],
                                             pt1[:, qo:qo + 128], tri_sb[:])
                    nc.tensor.matmul(psa0[:, qo:],
                                     v_v[:, t, 65 * h0:65 * (h0 + 1)],
                                     pt0[:, qo:], start=(t == 0),
                                     stop=(t == nt - 1))
                    nc.tensor.matmul(psa1[:, qo:],
                                     v_v[:, t, 65 * h1:65 * (h1 + 1)],
                                     pt1[:, qo:], start=(t == 0),
                                     stop=(t == nt - 1))
                for h, psa, koff in ((h0, psa0, 0), (h1, psa1, 64)):
                    d1 = den1_p.tile([65, 512], F32, tag="d1", name="d1")
                    nc.vector.tensor_copy(d1[64:65, :], psa[64:65, :])
                    nc.scalar.dma_start(out=den_t[h:h + 1, :],
                                        in_=d1[64:65, :])
                    if koff == 0:
                        nc.vector.tensor_copy(attnTb_t[krt][0:64, :],
                                              psa[0:64, :])
                    else:
                        t64 = tmp64_p.tile([64, 512], BF16, name="t64")
                        nc.vector.tensor_copy(t64[:], psa[0:64, :])
                        nc.scalar.dma_start(out=attnTb_t[krt][64:128, :],
                                            in_=t64[:])
                yield

            rec_t = rec_p.tile([8, 512], F32, name="rec")
            nc.vector.reciprocal_approx_fast(rec_t[:], den_t[:])
            for krt in range(4):
                rb = psL.tile([128, 512], F32, tag="psl", name="rb")
                nc.tensor.matmul(rb[:], sel_t[krt][:], rec_t[:],
                                 start=True, stop=True)
                nc.vector.tensor_mul(attnTb_t[krt][:], attnTb_t[krt][:],
                                     rb[:])
            yield

            # ---- attention projection -> rs_in block ----
            blk = c % 2
            ri = rs_in[c // 2][:] \
                .rearrange("(k i p) q -> k i p q", k=2, p=128)
            for cc in range(2):
                for ct in range(4):
                    dct = 4 * cc + ct
                    ps = psM.tile([128, 512], F32, tag="mm", name="ps_pr")
                    for r in range(4):
                        nc.tensor.matmul(
                            ps[:],
                            wpr_t[4 * cc + r][:, 128 * ct:128 * (ct + 1)],
                            attnTb_t[r][:], start=(r == 0), stop=(r == 3))
                    ai = ai_p.tile([128, 512], BF16, name="ai")
                    nc.vector.tensor_copy(ai[:], ps[:])
                    nc.sync.dma_start(out=ri[blk, dct], in_=ai[:])
                yield

        # ============== B: full-width MLP for owned chunk k ================
        def B(k):
            ro = rs_out[k][:].rearrange("(i p) q -> i p q", p=128)
            t1_t = []
            for i in range(DT):
                rob = rob_p.tile([128, 512], BF16, name="rob")
                nc.sync.dma_start(out=rob[:], in_=ro[i])
                xf2 = xf2_p.tile([128, 512], F32, name="xf2")
                nc.sync.dma_start(out=xf2[:], in_=xo[DT * k + i])
                t1 = t1_p.tile([128, 512], F32, name="t1")
                nc.vector.tensor_add(t1[:], rob[:], xf2[:])
                if use_projb:
                    nc.vector.tensor_scalar_add(t1[:], t1[:],
                                                projb_sb[:, i:i + 1])
                t1_t.append(t1)
            yield

            nTb_t = layernorm_to_bf16(t1_t, g1_sb, b1_sb, use_g1b1)
            yield

            # ---- fc + gelu ----
            gT_t = []
            for fg in range(8):
                wfh = []
                for half in range(2):
                    t = wf_p.tile([128, 4 * 512], BF16, name="wfh")
                    nc.sync.dma_start(out=t[:], in_=wfc[fg, half])
                    wfh.append(t)
                for ct in range(4):
                    f = 4 * fg + ct
                    ps = psM.tile([128, 512], F32, tag="mm", name="ps_fc")
                    for d in range(DT):
                        w = wfh[d // 4]
                        dd = d % 4
                        nc.tensor.matmul(
                            ps[:],
                            w[:, 512 * dd + 128 * ct:512 * dd + 128 * (ct + 1)],
                            nTb_t[d][:], start=(d == 0), stop=(d == DT - 1))
                    gt = gT_p.tile([128, 512], BF16, name="gt")
                    nc.scalar.activation(gt[:], ps[:], AF.Gelu_apprx_tanh,
                                         bias=fcb_sb[:, f:f + 1])
                    gT_t.append(gt)
                yield

            # ---- cproj (full width; contraction over all 32 f-tiles),
            # with LN2 stats emitted incrementally per output pair ----
            mar_t = []
            pA2, pB2 = ln_stat_begin(psS, "ps_s")
            for p4 in range(4):
                wcq = []
                for qtr in range(4):
                    t = wc_p.tile([128, 8 * 256], BF16, name="wcq")
                    nc.sync.dma_start(out=t[:], in_=wcp[p4, qtr])
                    wcq.append(t)
                for ci in range(2):
                    dct = 2 * p4 + ci
                    ps = psM.tile([128, 512], F32, tag="mm", name="ps_cp")
                    for f in range(FT):
                        w = wcq[f // 8]
                        fi = f % 8
                        nc.tensor.matmul(
                            ps[:],
                            w[:, 256 * fi + 128 * ci:256 * fi + 128 * (ci + 1)],
                            gT_t[f][:], start=(f == 0), stop=(f == FT - 1))
                    m2 = t1_p.tile([128, 512], F32, name="t1")
                    nc.vector.tensor_add(m2[:], ps[:], nTb_t[dct][:])
                    if use_cprojb:
                        nc.vector.tensor_scalar_add(
                            m2[:], m2[:], cprojb_sb[:, dct:dct + 1])
                    mar_t.append(m2)
                    ln_stat_tile(pA2, pB2, m2, dct)
                yield

            # ---- LN2 -> output ----
            rstd_ps, urstd_ps = ln_stat_finish(pA2, pB2, psA, "psa")
            for i in range(DT):
                ht = hT_p.tile([128, 512], F32, tag="hT", name="ht")
                nc.vector.tensor_mul(ht[:], mar_t[i][:], rstd_ps[:])
                nc.vector.tensor_sub(ht[:], ht[:], urstd_ps[:])
                if use_g2b2:
                    nc.vector.tensor_scalar(ht[:], ht[:], g2_sb[:, i:i + 1],
                                            b2_sb[:, i:i + 1],
                                            OP.mult, OP.add)
                nc.scalar.dma_start(out=out[DT * k + i], in_=ht[:])
            yield

        # ---- emission ----
        def run(g):
            for _ in g:
                pass

        def il(ga, gb, gb_limit=None):
            """Round-robin ga/gb; advance gb at most gb_limit steps, then
            finish ga.  Returns gb (possibly unfinished)."""
            steps = 0
            done_a = done_b = False
            while not (done_a and done_b):
                if not done_a:
                    try:
                        next(ga)
                    except StopIteration:
                        done_a = True
                if not done_b:
                    if gb_limit is not None and steps >= gb_limit:
                        done_b = True
                    else:
                        try:
                            next(gb)
                            steps += 1
                        except StopIteration:
                            done_b = True
            return gb

        def trigger_rs(j):
            nc.gpsimd.collective_compute(
                "ReduceScatter", OP.add, replica_groups=PAIRS,
                ins=[rs_in[j][:].opt()], outs=[rs_out[j][:].opt()])

        def A3_then_rs():
            yield from A(3)
            trigger_rs(1)

        import os
        sched = os.environ.get("K_SCHED", "full")
        run(A(0, xh0))
        run(A(1))
        trigger_rs(0)
        run(A(2))
        if sched == "serial":
            run(A3_then_rs())
            run(B(0))
            run(B(1))
        elif sched == "o1":
            b0 = B(0)
            il(A3_then_rs(), b0, gb_limit=2)
            run(b0)
            run(B(1))
        else:
            # explicit schedule: B's PE-visible work always sits behind
            # the A3 matmuls in the PE FIFO so a late RS never blocks it.
            a3 = A3_then_rs()
            b0 = B(0)
            next(a3)
            next(a3)          # A3 qkv cc0/cc1
            next(b0)          # B0 t1 loads+adds (no PE work)
            for _ in range(5):
                next(a3)      # A3 V, krt0..3
            next(b0)          # B0 LN1 (PE reaches it well after RS0)
            run(a3)           # A3 rec, proj, trigger RS1
            b1 = B(1)
            next(b0)          # fc fg0
            next(b1)          # B1 t1 loads+adds (no PE work)
            for _ in range(7):
                next(b0)      # fc fg1..7
            next(b1)          # B1 LN1 (after RS1 done)
            run(b0)           # cproj + LN2
            run(b1)           # fc, cproj, LN2

    nc.compile()
    return nc


_cache = {}


def _get_program(flags):
    if flags not in _cache:
        _cache[flags] = _build(*flags)
    return _cache[flags]


def _pack(a, rows, cols):
    """[R, C] -> [R//rows * C//cols, rows, cols], row-tile-major."""
    R, C = a.shape
    return np.ascontiguousarray(
        a.reshape(R // rows, rows, C // cols, cols).transpose(0, 2, 1, 3)
        .reshape(-1, rows, cols))


def _swap(p, n_rt, n_ct):
    """_pack gives (row-tile, col-tile) order; swap to (col, row)."""
    t = p.reshape(n_rt, n_ct, p.shape[1], p.shape[2])
    return np.ascontiguousarray(
        t.transpose(1, 0, 2, 3).reshape(-1, p.shape[1], p.shape[2]))


def _prepare_inputs(inputs):
    x = np.asarray(inputs["x"], np.float32)
    weight = float(np.asarray(inputs["weight"]).reshape(-1)[0])
    linear_w = np.asarray(inputs["linear_w"], np.float32)
    linear_b = np.asarray(inputs["linear_b"], np.float32)
    proj_w = np.asarray(inputs["proj_w"], np.float32)
    proj_b = np.asarray(inputs["proj_b"], np.float32)
    ln1_g = np.asarray(inputs["ln1_g"], np.float32)
    ln1_b = np.asarray(inputs["ln1_b"], np.float32)
    fc_w = np.asarray(inputs["fc_w"], np.float32)
    fc_b = np.asarray(inputs["fc_b"], np.float32)
    cproj_w = np.asarray(inputs["cproj_w"], np.float32)
    cproj_b = np.asarray(inputs["cproj_b"], np.float32)
    ln2_g = np.asarray(inputs["ln2_g"], np.float32)
    ln2_b = np.asarray(inputs["ln2_b"], np.float32)
    idx = np.asarray(inputs["idx"]).astype(np.int64).reshape(-1)
    forcing = bool(np.asarray(inputs["attn_forcing"]).reshape(-1)[0])

    flags = (
        bool(linear_b[:2048].any()),      # use_bqk
        bool(linear_b[2048:].any()),      # use_bv
        bool(proj_b.any()),
        bool(cproj_b.any()),
        bool((ln1_g != 1).any() or ln1_b.any()),
        bool((ln2_g != 1).any() or ln2_b.any()),
    )

    if forcing:
        lnw = float(np.log(weight)) if weight >= 1e-37 else -1e9
        pos = np.arange(S)
        lna_all = np.where(pos[None, :] >= idx[:, None], lnw, 0.0) \
            .astype(np.float32)
    else:
        lna_all = np.zeros((B, S), np.float32)

    tri_np = np.triu(np.ones((128, 128), np.float32)).astype(BF)
    sel_np = np.zeros((4, 8, 128), np.float32)
    for krt in range(4):
        sel_np[krt, 2 * krt, 0:64] = 1.0
        sel_np[krt, 2 * krt + 1, 64:128] = 1.0

    # ---- global (all-core) MLP weights ----
    # wfc tile (fg, d) of [128,512]; regroup free dim as (d, q) halves
    wfc_p = _swap(_pack(fc_w.astype(BF), 128, 512), DT, 8)   # (fg, d)
    wfc_g = np.ascontiguousarray(
        wfc_p.reshape(8, 2, 4, 128, 512).transpose(0, 1, 3, 2, 4)
        .reshape(8, 2, 128, 4 * 512))
    # wcp tile (p4, f) of [128,256]; quarters of 8 f-tiles
    wcp_p = _swap(_pack(cproj_w.astype(BF), 128, 256), FT, 4)  # (p4, f)
    wcp_g = np.ascontiguousarray(
        wcp_p.reshape(4, 4, 8, 128, 256).transpose(0, 1, 3, 2, 4)
        .reshape(4, 4, 128, 8 * 256))

    in_maps = []
    for core in range(N_CORES):
        b, r = core // 2, core % 2
        q_cols = slice(512 * r, 512 * (r + 1))
        k_cols = slice(1024 + 512 * r, 1024 + 512 * (r + 1))
        v_cols = slice(2048 + 512 * r, 2048 + 512 * (r + 1))
        xT = np.ascontiguousarray(x[b].T)                       # [D, S]
        wqk_full = np.concatenate([linear_w[:, q_cols], linear_w[:, k_cols]],
                                  axis=1)                       # [D, 1024]

        xq_t = _pack(xT, 128, 512)                  # (d, c): index d*NCH+c
        xq_dc = xq_t.reshape(DT, NCH, 128, 512)
        # xqb[c, half] = [128, (d%4, q)] bf16
        xqb = np.ascontiguousarray(
            xq_dc.transpose(1, 0, 2, 3).reshape(NCH, 2, 4, 128, 512)
            .transpose(0, 1, 3, 2, 4).reshape(NCH, 2, 128, 4 * 512)
        ).astype(BF)
        own = [r, 2 + r]
        xo_np = np.ascontiguousarray(
            xq_dc[:, own].transpose(1, 0, 2, 3).reshape(2 * DT, 128, 512))

        in_maps.append({
            "xqb": xqb,
            "xo": xo_np,
            "wqk": _swap(_pack(wqk_full.astype(BF), 128, 512), 8, 2),
            "bqk": np.ascontiguousarray(
                np.concatenate([linear_b[q_cols], linear_b[k_cols]])),
            "wv": _pack(linear_w[:, v_cols].astype(BF), 128, 512),
            "bv": np.ascontiguousarray(linear_b[v_cols]).astype(BF),
            "wproj": _swap(_pack(proj_w[512 * r:512 * (r + 1), :].astype(BF),
                                 128, 512), 4, 2),
            "projb": proj_b,
            "wfc": wfc_g, "fcb": fc_b,
            "wcp": wcp_g, "cprojb": cproj_b,
            "g1": ln1_g, "b1": ln1_b, "g2": ln2_g, "b2": ln2_b,
            "lna": lna_all[b],
            "tri": tri_np,
            "sel": sel_np,
        })
    return flags, in_maps


def _unpack_out(o_pair):
    """o_pair: [out_core0, out_core1] each [2*DT, 128, 512] -> [S, D]."""
    hT = np.empty((D, S), np.float32)
    for r in range(2):
        o = o_pair[r].reshape(2, DT, 128, 512)
        for k, c in enumerate([r, 2 + r]):
            hT[:, CH * c:CH * (c + 1)] = o[k].reshape(D, 512)
    return hT.T


def _run(inputs, trace=False):
    flags, in_maps = _prepare_inputs(inputs)
    nc = _get_program(flags)
    r = run_bass_kernel_spmd(nc, in_maps, core_ids=list(range(N_CORES)),
                             trace=trace)
    outs = np.stack(
        [_unpack_out([r.results[2 * b]["out"], r.results[2 * b + 1]["out"]])
         for b in range(B)], axis=0).astype(np.float32)
    return outs, r


def kernel(**inputs) -> np.ndarray:
    outs, _ = _run(inputs, trace=False)
    return outs
